# revision 13
# baseline (speedup 1.0000x reference)
"""MoE (top-2 of 8 experts, SwiGLU) Trainium2 kernel — expert-parallel over 8 NeuronCores.

Strategy
--------
- Host computes the tiny router (T x D @ D x 8 = 0.03% of total FLOPs) with the
  exact same jax ops as the reference, so top-k selection matches bitwise.
- Token dispatch ("all-to-all") happens on the host: tokens routed to expert e
  are gathered (transposed, capacity-padded) and shipped to core e.
- Each of the 8 cores runs an identical SPMD Bass program: the SwiGLU expert
  FFN for its expert over its capacity-C token slab.  Matmuls use float32r
  (full fp32 data; streams at 1 cycle/row for moving dim >= 256, i.e. bf16
  speed with fp32-grade precision).
- Host scatter-adds the two expert outputs per token back together with the
  routing weights (exactly the reference's dense-combine math restricted to
  the nonzero entries).

Per-core device work: ~C x (3 matmuls of [*,1024]x[1024,1024]) ≈ 2.6e10 FLOPs
(sparse: 4x less than the reference's dense form), vs 1.03e11 dense.
"""

import math
from contextlib import ExitStack

import numpy as np

import concourse.bass as bass
import concourse.tile as tile
from concourse import bacc, mybir
from concourse.bass_utils import run_bass_kernel_spmd

D = 1024       # model dim
H = 1024       # per-expert hidden dim
E = 8          # experts == cores
TG = 512       # tokens per tile chunk (full PSUM bank; float32r full rate >= 256)
MM_DT = mybir.dt.float32r
ACC_DT = mybir.dt.float32

_BUILD_CACHE: dict[int, object] = {}


def _chunks(C):
    """Split C tokens into 512-token chunks plus one ragged 128-multiple
    chunk, placed FIRST: the smaller first x-transfer lets PE start sooner."""
    out = [TG] * (C // TG)
    if C % TG:
        out.insert(0, C % TG)
    return out


def _build(C):
    """Build + compile the single-expert SwiGLU FFN program for capacity C.

    Computes yt = (silu(xt.T @ pw) * (xt.T @ gw)) @ ow, transposed:
    everything is laid out [feature, token] so no on-device transposes are
    needed (host ships x pre-transposed and un-transposes y).
    """
    assert C % 128 == 0
    nc = bacc.Bacc("TRN2", target_bir_lowering=False, debug=False, num_devices=E)
    xt = nc.dram_tensor("xt", [D, C], MM_DT, kind="ExternalInput").ap()
    gw = nc.dram_tensor("gw", [D, H], MM_DT, kind="ExternalInput").ap()
    pw = nc.dram_tensor("pw", [D, H], MM_DT, kind="ExternalInput").ap()
    ow = nc.dram_tensor("ow", [H, D], MM_DT, kind="ExternalInput").ap()
    yt = nc.dram_tensor("yt", [D, C], ACC_DT, kind="ExternalOutput").ap()

    KB = D // 128  # contraction blocks for the first matmuls
    HB = H // 128  # hidden blocks

    # Partition-blocked 3D views: [(a p) m] -> [p, a, m] so each weight
    # matrix / token chunk moves as ONE large DMA (HWDGE trigger is ~600ns
    # of engine time each; batching to >=1MiB is the documented rule).
    xt_r = xt.rearrange("(a p) c -> p a c", p=128)
    yt_r = yt.rearrange("(a p) c -> p a c", p=128)
    gw_r = gw.rearrange("(a p) m -> p a m", p=128)
    pw_r = pw.rearrange("(a p) m -> p a m", p=128)
    ow_r = ow.rearrange("(a p) m -> p a m", p=128)

    with tile.TileContext(nc) as tc, ExitStack() as ctx:
        wpool = ctx.enter_context(tc.tile_pool(name="w", bufs=1))
        xpool = ctx.enter_context(tc.tile_pool(name="x", bufs=2))
        hpool = ctx.enter_context(tc.tile_pool(name="h", bufs=2))
        spool = ctx.enter_context(tc.tile_pool(name="s", bufs=2))
        ypool = ctx.enter_context(tc.tile_pool(name="y", bufs=2))
        gpsum = ctx.enter_context(tc.tile_pool(name="pg", bufs=3, space="PSUM"))
        ppsum = ctx.enter_context(tc.tile_pool(name="pp", bufs=3, space="PSUM"))
        ypsum = ctx.enter_context(tc.tile_pool(name="py", bufs=2, space="PSUM"))

        def load_x(col, tg):
            t = xpool.tile([128, KB, tg], MM_DT, tag="x")
            nc.sync.dma_start(t[:], xt_r[:, :, col:col + tg])
            return t

        def load_w(ap_r, prefix):
            # two halves: G can start after the first 4 k-blocks land, and
            # the trigger pipeline (one HWDGE ring) interleaves better.
            t = wpool.tile([128, KB, H], MM_DT, tag=prefix)
            half = KB // 2
            nc.sync.dma_start(t[:, :half, :], ap_r[:, :half, :])
            nc.sync.dma_start(t[:, half:, :], ap_r[:, half:, :])
            return t

        # Issue order matters: PE starts on chunk 0's G matmuls as soon as
        # x(chunk0) + gw arrive; pw is needed ~14us later, ow ~28us later
        # (and the Y phase runs one chunk behind to hide ow's wire time).
        chunk_list = _chunks(C)
        xs_next = load_x(0, chunk_list[0])
        gw_t = load_w(gw_r, "gw")
        pw_t = load_w(pw_r, "pw")
        ow_t = load_w(ow_r, "ow")

        pending_y = None  # (hs, col, tg) of the previous chunk

        def emit_y(hs, ycol, tg):
            ybig = ypool.tile([128, KB, tg], ACC_DT, tag="y")
            for dblk in range(KB):
                py = ypsum.tile([128, tg], ACC_DT, tag="py")
                for h in range(HB):
                    nc.tensor.matmul(
                        py[:], ow_t[:, h, dblk * 128:(dblk + 1) * 128], hs[h][:],
                        start=(h == 0), stop=(h == HB - 1))
                nc.vector.tensor_copy(ybig[:, dblk, :], py[:])
            # ACT's HWDGE ring: keep SyncE's ring free for input feeding
            nc.scalar.dma_start(yt_r[:, :, ycol:ycol + tg], ybig[:])

        col = 0
        for ci, tg in enumerate(chunk_list):
            xs = xs_next
            if ci + 1 < len(chunk_list):
                xs_next = load_x(col + tg, chunk_list[ci + 1])

            hs = []
            for h in range(HB):
                pg = gpsum.tile([128, tg], ACC_DT, tag="pg")
                for k in range(KB):
                    nc.tensor.matmul(
                        pg[:], gw_t[:, k, h * 128:(h + 1) * 128], xs[:, k, :],
                        start=(k == 0), stop=(k == KB - 1))
                pp = ppsum.tile([128, tg], ACC_DT, tag="pp")
                for k in range(KB):
                    nc.tensor.matmul(
                        pp[:], pw_t[:, k, h * 128:(h + 1) * 128], xs[:, k, :],
                        start=(k == 0), stop=(k == KB - 1))
                sg = spool.tile([128, tg], ACC_DT, tag="sig")
                nc.scalar.activation(sg[:], pp[:], mybir.ActivationFunctionType.Sigmoid)
                sl = spool.tile([128, tg], ACC_DT, tag="sil")
                nc.vector.tensor_mul(sl[:], pp[:], sg[:])   # silu(p), one PSUM read
                ht = hpool.tile([128, tg], MM_DT, tag=f"h{h}")
                nc.vector.tensor_mul(ht[:], pg[:], sl[:])
                hs.append(ht)

            if pending_y is not None:
                emit_y(*pending_y)
            pending_y = (hs, col, tg)
            col += tg
        emit_y(*pending_y)

    nc.compile()
    return nc


def _get_program(C):
    if C not in _BUILD_CACHE:
        _BUILD_CACHE[C] = _build(C)
    return _BUILD_CACHE[C]


def _route(x, gate_w, k):
    """Router with the reference's exact jax ops (bitwise-matching top-k)."""
    import jax
    import jax.numpy as jnp

    router_logits = jnp.asarray(x) @ jnp.asarray(gate_w)
    routing_probs = jax.nn.softmax(router_logits.astype(jnp.float32), axis=-1)
    top_w, top_i = jax.lax.top_k(routing_probs, k)
    top_w = top_w / jnp.sum(top_w, axis=-1, keepdims=True)
    top_w = top_w.astype(jnp.float32)

    n_exp = gate_w.shape[1]
    expert_mask = jax.nn.one_hot(top_i, n_exp, dtype=jnp.float32)
    tokens_per_expert = jnp.mean(expert_mask, axis=0)
    router_prob_per_expert = jnp.mean(routing_probs, axis=0)
    bl_loss = jnp.sum(tokens_per_expert * router_prob_per_expert[None, :]) * n_exp

    return (np.asarray(router_logits), np.asarray(top_i), np.asarray(top_w),
            np.asarray(bl_loss))


def _run_device(nc, in_maps, trace=False, **kw):
    return run_bass_kernel_spmd(nc, in_maps, core_ids=list(range(E)),
                                trace=trace, **kw)


def kernel(hidden_states, gate_w, gw, pw, ow, top_k, _trace=False, _res_out=None):
    hidden_states = np.asarray(hidden_states, dtype=np.float32)
    gate_w = np.asarray(gate_w, dtype=np.float32)
    gw = np.ascontiguousarray(np.asarray(gw, dtype=np.float32))
    pw = np.ascontiguousarray(np.asarray(pw, dtype=np.float32))
    ow = np.ascontiguousarray(np.asarray(ow, dtype=np.float32))
    k = int(top_k)

    B, S, _ = hidden_states.shape
    x = hidden_states.reshape(-1, D)
    T = x.shape[0]

    router_logits, top_i, top_w, bl_loss = _route(x, gate_w, k)

    # --- host dispatch (the "all-to-all") ---
    counts = np.bincount(top_i.ravel(), minlength=E)
    C = max(128, int(math.ceil(counts.max() / 128)) * 128)
    prog = _get_program(C)

    xT = np.ascontiguousarray(x.T)  # [D, T] so per-expert gathers are column slices
    idx_list, w_list, in_maps = [], [], []
    for e in range(E):
        sel = top_i == e                       # [T, k]
        idx = np.nonzero(sel.any(axis=1))[0]   # tokens routed to e
        wgt = (top_w[idx] * sel[idx]).sum(axis=1).astype(np.float32)
        xt = np.zeros((D, C), np.float32)
        xt[:, :idx.size] = xT[:, idx]
        idx_list.append(idx)
        w_list.append(wgt)
        in_maps.append({"xt": xt, "gw": gw[e], "pw": pw[e], "ow": ow[e]})

    res = _run_device(prog, in_maps, trace=_trace)
    if _res_out is not None:
        _res_out.append(res)

    final = np.zeros((T, D), np.float32)
    for e in range(E):
        idx = idx_list[e]
        ye = res.results[e]["yt"][:, :idx.size].T  # [n_e, D]
        final[idx] += ye * w_list[e][:, None]

    return (final.reshape(B, S, D),
            router_logits,
            np.float32(bl_loss))


# revision 14
# speedup vs baseline: 1.1891x; 1.1891x over previous
"""MoE (top-2 of 8 experts, SwiGLU) Trainium2 kernel — expert-parallel over 8 NeuronCores.

Strategy
--------
- Host computes the tiny router (T x D @ D x 8 = 0.03% of total FLOPs) with the
  exact same jax ops as the reference, so top-k selection matches bitwise.
- Token dispatch ("all-to-all") happens on the host: tokens routed to expert e
  are gathered (transposed, capacity-padded) and shipped to core e.
- Each of the 8 cores runs an identical SPMD Bass program: the SwiGLU expert
  FFN for its expert over its capacity-C token slab.  Matmuls use float32r
  (full fp32 data; streams at 1 cycle/row for moving dim >= 256, i.e. bf16
  speed with fp32-grade precision).
- Host scatter-adds the two expert outputs per token back together with the
  routing weights (exactly the reference's dense-combine math restricted to
  the nonzero entries).

Per-core device work: ~C x (3 matmuls of [*,1024]x[1024,1024]) ≈ 2.6e10 FLOPs
(sparse: 4x less than the reference's dense form), vs 1.03e11 dense.
"""

import math
from contextlib import ExitStack

import numpy as np

import concourse.bass as bass
import concourse.tile as tile
from concourse import bacc, mybir
from concourse.bass_utils import run_bass_kernel_spmd

D = 1024       # model dim
H = 1024       # per-expert hidden dim
E = 8          # experts == cores
TG = 512       # tokens per tile chunk (full PSUM bank; float32r full rate >= 256)
MM_DT = mybir.dt.float32r
ACC_DT = mybir.dt.float32

_BUILD_CACHE: dict[int, object] = {}


def _chunks(C):
    """Split C tokens into chunks of 512 plus one ragged 128-multiple tail."""
    out = [TG] * (C // TG)
    if C % TG:
        out.append(C % TG)
    return out


def _build(C):
    """Build + compile the single-expert SwiGLU FFN program for capacity C.

    Computes yt = (silu(xt.T @ pw) * (xt.T @ gw)) @ ow, transposed:
    everything is laid out [feature, token] so no on-device transposes are
    needed (host ships x pre-transposed and un-transposes y).
    """
    assert C % 128 == 0
    nc = bacc.Bacc("TRN2", target_bir_lowering=False, debug=False, num_devices=E)
    xt = nc.dram_tensor("xt", [D, C], MM_DT, kind="ExternalInput").ap()
    gw = nc.dram_tensor("gw", [D, H], MM_DT, kind="ExternalInput").ap()
    pw = nc.dram_tensor("pw", [D, H], MM_DT, kind="ExternalInput").ap()
    ow = nc.dram_tensor("ow", [H, D], MM_DT, kind="ExternalInput").ap()
    yt = nc.dram_tensor("yt", [D, C], ACC_DT, kind="ExternalOutput").ap()

    KB = D // 128  # contraction blocks for the first matmuls
    HB = H // 128  # hidden blocks

    # Partition-blocked 3D views: [(a p) m] -> [p, a, m] so each weight
    # matrix / token chunk moves as ONE large DMA (HWDGE trigger is ~600ns
    # of engine time each; batching to >=1MiB is the documented rule).
    xt_r = xt.rearrange("(a p) c -> p a c", p=128)
    yt_r = yt.rearrange("(a p) c -> p a c", p=128)
    gw_r = gw.rearrange("(a p) m -> p a m", p=128)
    pw_r = pw.rearrange("(a p) m -> p a m", p=128)
    ow_r = ow.rearrange("(a p) m -> p a m", p=128)

    with tile.TileContext(nc) as tc, ExitStack() as ctx:
        wpool = ctx.enter_context(tc.tile_pool(name="w", bufs=1))
        xpool = ctx.enter_context(tc.tile_pool(name="x", bufs=2))
        hpool = ctx.enter_context(tc.tile_pool(name="h", bufs=2))
        spool = ctx.enter_context(tc.tile_pool(name="s", bufs=2))
        ypool = ctx.enter_context(tc.tile_pool(name="y", bufs=2))
        gpsum = ctx.enter_context(tc.tile_pool(name="pg", bufs=3, space="PSUM"))
        ppsum = ctx.enter_context(tc.tile_pool(name="pp", bufs=3, space="PSUM"))
        ypsum = ctx.enter_context(tc.tile_pool(name="py", bufs=2, space="PSUM"))

        def load_x(col, tg):
            t = xpool.tile([128, KB, tg], MM_DT, tag="x")
            nc.sync.dma_start(t[:], xt_r[:, :, col:col + tg])
            return t

        def load_w(ap_r, prefix):
            # two halves: G can start after the first 4 k-blocks land, and
            # the trigger pipeline (one HWDGE ring) interleaves better.
            t = wpool.tile([128, KB, H], MM_DT, tag=prefix)
            half = KB // 2
            nc.sync.dma_start(t[:, :half, :], ap_r[:, :half, :])
            nc.sync.dma_start(t[:, half:, :], ap_r[:, half:, :])
            return t

        # Issue order matters: PE starts on chunk 0's G matmuls as soon as
        # x(chunk0) + gw arrive; pw is needed ~14us later, ow ~28us later
        # (and the Y phase runs one chunk behind to hide ow's wire time).
        chunk_list = _chunks(C)
        xs_next = load_x(0, chunk_list[0])
        gw_t = load_w(gw_r, "gw")
        pw_t = load_w(pw_r, "pw")
        ow_t = load_w(ow_r, "ow")

        pending_y = None  # (hs, col, tg) of the previous chunk

        def emit_y(hs, ycol, tg):
            ybig = ypool.tile([128, KB, tg], ACC_DT, tag="y")
            for dblk in range(KB):
                py = ypsum.tile([128, tg], ACC_DT, tag="py")
                for h in range(HB):
                    nc.tensor.matmul(
                        py[:], ow_t[:, h, dblk * 128:(dblk + 1) * 128], hs[h][:],
                        start=(h == 0), stop=(h == HB - 1))
                nc.vector.tensor_copy(ybig[:, dblk, :], py[:])
            # ACT's HWDGE ring: keep SyncE's ring free for input feeding
            nc.scalar.dma_start(yt_r[:, :, ycol:ycol + tg], ybig[:])

        col = 0
        for ci, tg in enumerate(chunk_list):
            xs = xs_next
            if ci + 1 < len(chunk_list):
                xs_next = load_x(col + tg, chunk_list[ci + 1])

            hs = []
            for h in range(HB):
                pg = gpsum.tile([128, tg], ACC_DT, tag="pg")
                for k in range(KB):
                    nc.tensor.matmul(
                        pg[:], gw_t[:, k, h * 128:(h + 1) * 128], xs[:, k, :],
                        start=(k == 0), stop=(k == KB - 1))
                pp = ppsum.tile([128, tg], ACC_DT, tag="pp")
                for k in range(KB):
                    nc.tensor.matmul(
                        pp[:], pw_t[:, k, h * 128:(h + 1) * 128], xs[:, k, :],
                        start=(k == 0), stop=(k == KB - 1))
                sg = spool.tile([128, tg], ACC_DT, tag="sig")
                nc.scalar.activation(sg[:], pp[:], mybir.ActivationFunctionType.Sigmoid)
                sl = spool.tile([128, tg], ACC_DT, tag="sil")
                nc.vector.tensor_mul(sl[:], pp[:], sg[:])   # silu(p), one PSUM read
                ht = hpool.tile([128, tg], MM_DT, tag=f"h{h}")
                nc.vector.tensor_mul(ht[:], pg[:], sl[:])
                hs.append(ht)

            if pending_y is not None:
                emit_y(*pending_y)
            pending_y = (hs, col, tg)
            col += tg
        emit_y(*pending_y)

    nc.compile()
    return nc


def _get_program(C):
    if C not in _BUILD_CACHE:
        _BUILD_CACHE[C] = _build(C)
    return _BUILD_CACHE[C]


def _route(x, gate_w, k):
    """Router with the reference's exact jax ops (bitwise-matching top-k)."""
    import jax
    import jax.numpy as jnp

    router_logits = jnp.asarray(x) @ jnp.asarray(gate_w)
    routing_probs = jax.nn.softmax(router_logits.astype(jnp.float32), axis=-1)
    top_w, top_i = jax.lax.top_k(routing_probs, k)
    top_w = top_w / jnp.sum(top_w, axis=-1, keepdims=True)
    top_w = top_w.astype(jnp.float32)

    n_exp = gate_w.shape[1]
    expert_mask = jax.nn.one_hot(top_i, n_exp, dtype=jnp.float32)
    tokens_per_expert = jnp.mean(expert_mask, axis=0)
    router_prob_per_expert = jnp.mean(routing_probs, axis=0)
    bl_loss = jnp.sum(tokens_per_expert * router_prob_per_expert[None, :]) * n_exp

    return (np.asarray(router_logits), np.asarray(top_i), np.asarray(top_w),
            np.asarray(bl_loss))


def _run_device(nc, in_maps, trace=False, **kw):
    return run_bass_kernel_spmd(nc, in_maps, core_ids=list(range(E)),
                                trace=trace, **kw)


def kernel(hidden_states, gate_w, gw, pw, ow, top_k, _trace=False, _res_out=None):
    hidden_states = np.asarray(hidden_states, dtype=np.float32)
    gate_w = np.asarray(gate_w, dtype=np.float32)
    gw = np.ascontiguousarray(np.asarray(gw, dtype=np.float32))
    pw = np.ascontiguousarray(np.asarray(pw, dtype=np.float32))
    ow = np.ascontiguousarray(np.asarray(ow, dtype=np.float32))
    k = int(top_k)

    B, S, _ = hidden_states.shape
    x = hidden_states.reshape(-1, D)
    T = x.shape[0]

    router_logits, top_i, top_w, bl_loss = _route(x, gate_w, k)

    # --- host dispatch (the "all-to-all") ---
    counts = np.bincount(top_i.ravel(), minlength=E)
    C = max(128, int(math.ceil(counts.max() / 128)) * 128)
    prog = _get_program(C)

    xT = np.ascontiguousarray(x.T)  # [D, T] so per-expert gathers are column slices
    idx_list, w_list, in_maps = [], [], []
    for e in range(E):
        sel = top_i == e                       # [T, k]
        idx = np.nonzero(sel.any(axis=1))[0]   # tokens routed to e
        wgt = (top_w[idx] * sel[idx]).sum(axis=1).astype(np.float32)
        xt = np.zeros((D, C), np.float32)
        xt[:, :idx.size] = xT[:, idx]
        idx_list.append(idx)
        w_list.append(wgt)
        in_maps.append({"xt": xt, "gw": gw[e], "pw": pw[e], "ow": ow[e]})

    res = _run_device(prog, in_maps, trace=_trace)
    if _res_out is not None:
        _res_out.append(res)

    final = np.zeros((T, D), np.float32)
    for e in range(E):
        idx = idx_list[e]
        ye = res.results[e]["yt"][:, :idx.size].T  # [n_e, D]
        final[idx] += ye * w_list[e][:, None]

    return (final.reshape(B, S, D),
            router_logits,
            np.float32(bl_loss))


# revision 28
# speedup vs baseline: 1.2005x; 1.0096x over previous
"""MoE (top-2 of 8 experts, SwiGLU) Trainium2 kernel — expert-parallel over 8 NeuronCores.

Strategy
--------
- Host computes the tiny router (T x D @ D x 8 = 0.03% of total FLOPs) with the
  exact same jax ops as the reference, so top-k selection matches bitwise.
- Token dispatch ("all-to-all") happens on the host: tokens routed to expert e
  are gathered (transposed, capacity-padded) and shipped to core e.
- Each of the 8 cores runs an identical SPMD Bass program: the SwiGLU expert
  FFN for its expert over its capacity-C token slab.  Matmuls use float32r
  (full fp32 data; streams at 1 cycle/row for moving dim >= 256, i.e. bf16
  speed with fp32-grade precision).
- Host scatter-adds the two expert outputs per token back together with the
  routing weights (exactly the reference's dense-combine math restricted to
  the nonzero entries).

Per-core device work: ~C x (3 matmuls of [*,1024]x[1024,1024]) ≈ 2.6e10 FLOPs
(sparse: 4x less than the reference's dense form), vs 1.03e11 dense.
"""

import math
from contextlib import ExitStack

import numpy as np

import concourse.bass as bass
import concourse.tile as tile
from concourse import bacc, mybir
from concourse.bass_utils import run_bass_kernel_spmd

D = 1024       # model dim
H = 1024       # per-expert hidden dim
E = 8          # experts == cores
TG = 512       # tokens per tile chunk (full PSUM bank; float32r full rate >= 256)
MM_DT = mybir.dt.float32r   # moving operand (activations)
W_DT = mybir.dt.float32r    # stationary operand (weights; must match moving dtype)
ACC_DT = mybir.dt.float32

_BUILD_CACHE: dict[int, object] = {}


def _chunks(C):
    """Split C tokens into chunks of 512 plus one ragged 128-multiple tail."""
    out = [TG] * (C // TG)
    if C % TG:
        out.append(C % TG)
    return out


def _build(C):
    """Build + compile the single-expert SwiGLU FFN program for capacity C.

    Computes yt = (silu(xt.T @ pw) * (xt.T @ gw)) @ ow, transposed:
    everything is laid out [feature, token] so no on-device transposes are
    needed (host ships x pre-transposed and un-transposes y).
    """
    assert C % 128 == 0
    nc = bacc.Bacc("TRN2", target_bir_lowering=False, debug=False, num_devices=E)
    xt = nc.dram_tensor("xt", [D, C], MM_DT, kind="ExternalInput").ap()
    gw = nc.dram_tensor("gw", [D, H], W_DT, kind="ExternalInput").ap()
    pw = nc.dram_tensor("pw", [D, H], W_DT, kind="ExternalInput").ap()
    ow = nc.dram_tensor("ow", [H, D], W_DT, kind="ExternalInput").ap()
    yt = nc.dram_tensor("yt", [D, C], ACC_DT, kind="ExternalOutput").ap()

    KB = D // 128  # contraction blocks for the first matmuls
    HB = H // 128  # hidden blocks

    # Partition-blocked 3D views: [(a p) m] -> [p, a, m] so each weight
    # matrix / token chunk moves as ONE large DMA (HWDGE trigger is ~600ns
    # of engine time each; batching to >=1MiB is the documented rule).
    xt_r = xt.rearrange("(a p) c -> p a c", p=128)
    yt_r = yt.rearrange("(a p) c -> p a c", p=128)
    gw_r = gw.rearrange("(a p) m -> p a m", p=128)
    pw_r = pw.rearrange("(a p) m -> p a m", p=128)
    ow_r = ow.rearrange("(a p) m -> p a m", p=128)

    with tile.TileContext(nc) as tc, ExitStack() as ctx:
        wpool = ctx.enter_context(tc.tile_pool(name="w", bufs=1))
        xpool = ctx.enter_context(tc.tile_pool(name="x", bufs=2))
        hpool = ctx.enter_context(tc.tile_pool(name="h", bufs=2))
        spool = ctx.enter_context(tc.tile_pool(name="s", bufs=2))
        ypool = ctx.enter_context(tc.tile_pool(name="y", bufs=1))
        gpsum = ctx.enter_context(tc.tile_pool(name="pg", bufs=3, space="PSUM"))
        ppsum = ctx.enter_context(tc.tile_pool(name="pp", bufs=3, space="PSUM"))
        ypsum = ctx.enter_context(tc.tile_pool(name="py", bufs=2, space="PSUM"))

        def load_x(col, tg):
            t = xpool.tile([128, KB, tg], MM_DT, tag="x")
            nc.sync.dma_start(t[:], xt_r[:, :, col:col + tg])
            return t

        def load_w(ap_r, prefix):
            # two halves: G can start after the first 4 k-blocks land, and
            # the trigger pipeline (one HWDGE ring) interleaves better.
            t = wpool.tile([128, KB, H], W_DT, tag=prefix)
            half = KB // 2
            nc.sync.dma_start(t[:, :half, :], ap_r[:, :half, :])
            nc.sync.dma_start(t[:, half:, :], ap_r[:, half:, :])
            return t

        # Issue order matters: PE starts on chunk 0's G matmuls as soon as
        # x(chunk0) + gw arrive; pw is needed ~14us later, ow ~28us later
        # (and the Y phase runs one chunk behind to hide ow's wire time).
        chunk_list = _chunks(C)
        xs_next = load_x(0, chunk_list[0])
        gw_t = load_w(gw_r, "gw")
        pw_t = load_w(pw_r, "pw")
        ow_t = load_w(ow_r, "ow")

        pending_y = None  # (hs, col, tg) of the previous chunk

        def emit_y(hs, ycol, tg):
            ybig = ypool.tile([128, KB, tg], ACC_DT, tag="y")
            for dblk in range(KB):
                py = ypsum.tile([128, tg], ACC_DT, tag="py")
                for h in range(HB):
                    nc.tensor.matmul(
                        py[:], ow_t[:, h, dblk * 128:(dblk + 1) * 128], hs[h][:],
                        start=(h == 0), stop=(h == HB - 1))
                nc.vector.tensor_copy(ybig[:, dblk, :], py[:])
            # ACT's HWDGE ring: keep SyncE's ring free for input feeding
            nc.scalar.dma_start(yt_r[:, :, ycol:ycol + tg], ybig[:])

        def emit_gp_chunk0(xs, tg):
            """Chunk 0 with k-OUTER matmul order: h-outer needs the whole
            weight matrix within ~2us, but the wire delivers it over ~17us;
            k-outer consumes each 512KB k-tile right as it lands.  G results
            are copied out of PSUM to SBUF so P can reuse the banks (silu
            needs P of the same h before G could otherwise drain)."""
            hs = []
            g_sb = []
            # 4 concurrent accumulators per wave = 2 slots from the G/P pool
            # + 2 borrowed from the (not-yet-used) Y pool: stays within the
            # 8 PSUM banks without growing any pool.
            def wave_tiles(pool, tag):
                return [pool.tile([128, tg], ACC_DT, tag=tag, name=f"{tag}w0"),
                        pool.tile([128, tg], ACC_DT, tag=tag, name=f"{tag}w1"),
                        ypsum.tile([128, tg], ACC_DT, tag="py", name=f"{tag}w2"),
                        ypsum.tile([128, tg], ACC_DT, tag="py", name=f"{tag}w3")]
            for wave in range(2):          # h-blocks 0..3, then 4..7
                pgs = wave_tiles(gpsum, "pg")
                for k in range(KB):
                    for i in range(4):
                        h = wave * 4 + i
                        nc.tensor.matmul(
                            pgs[i][:], gw_t[:, k, h * 128:(h + 1) * 128],
                            xs[:, k, :], start=(k == 0), stop=(k == KB - 1))
                for i in range(4):
                    g = wpool.tile([128, tg], ACC_DT, tag=f"gsb{wave * 4 + i}")
                    nc.vector.tensor_copy(g[:], pgs[i][:])
                    g_sb.append(g)
            for wave in range(2):
                pps = wave_tiles(ppsum, "pp")
                for k in range(KB):
                    for i in range(4):
                        h = wave * 4 + i
                        nc.tensor.matmul(
                            pps[i][:], pw_t[:, k, h * 128:(h + 1) * 128],
                            xs[:, k, :], start=(k == 0), stop=(k == KB - 1))
                for i in range(4):
                    h = wave * 4 + i
                    sg = spool.tile([128, tg], ACC_DT, tag="sig")
                    nc.scalar.activation(
                        sg[:], pps[i][:], mybir.ActivationFunctionType.Sigmoid)
                    sl = spool.tile([128, tg], ACC_DT, tag="sil")
                    nc.vector.tensor_mul(sl[:], pps[i][:], sg[:])
                    ht = hpool.tile([128, tg], MM_DT, tag=f"h{h}")
                    nc.vector.tensor_mul(ht[:], g_sb[h][:], sl[:])
                    hs.append(ht)
            return hs

        def emit_gp(xs, tg):
            hs = []
            for h in range(HB):
                pg = gpsum.tile([128, tg], ACC_DT, tag="pg")
                for k in range(KB):
                    nc.tensor.matmul(
                        pg[:], gw_t[:, k, h * 128:(h + 1) * 128], xs[:, k, :],
                        start=(k == 0), stop=(k == KB - 1))
                pp = ppsum.tile([128, tg], ACC_DT, tag="pp")
                for k in range(KB):
                    nc.tensor.matmul(
                        pp[:], pw_t[:, k, h * 128:(h + 1) * 128], xs[:, k, :],
                        start=(k == 0), stop=(k == KB - 1))
                sg = spool.tile([128, tg], ACC_DT, tag="sig")
                nc.scalar.activation(sg[:], pp[:], mybir.ActivationFunctionType.Sigmoid)
                sl = spool.tile([128, tg], ACC_DT, tag="sil")
                nc.vector.tensor_mul(sl[:], pp[:], sg[:])   # silu(p), one PSUM read
                ht = hpool.tile([128, tg], MM_DT, tag=f"h{h}")
                nc.vector.tensor_mul(ht[:], pg[:], sl[:])
                hs.append(ht)
            return hs

        col = 0
        for ci, tg in enumerate(chunk_list):
            xs = xs_next
            if ci + 1 < len(chunk_list):
                xs_next = load_x(col + tg, chunk_list[ci + 1])

            hs = emit_gp_chunk0(xs, tg) if ci == 0 else emit_gp(xs, tg)

            if pending_y is not None:
                emit_y(*pending_y)
            pending_y = (hs, col, tg)
            col += tg
        emit_y(*pending_y)

    nc.compile()
    return nc


def _get_program(C):
    if C not in _BUILD_CACHE:
        _BUILD_CACHE[C] = _build(C)
    return _BUILD_CACHE[C]


def _route(x, gate_w, k):
    """Router with the reference's exact jax ops (bitwise-matching top-k)."""
    import jax
    import jax.numpy as jnp

    router_logits = jnp.asarray(x) @ jnp.asarray(gate_w)
    routing_probs = jax.nn.softmax(router_logits.astype(jnp.float32), axis=-1)
    top_w, top_i = jax.lax.top_k(routing_probs, k)
    top_w = top_w / jnp.sum(top_w, axis=-1, keepdims=True)
    top_w = top_w.astype(jnp.float32)

    n_exp = gate_w.shape[1]
    expert_mask = jax.nn.one_hot(top_i, n_exp, dtype=jnp.float32)
    tokens_per_expert = jnp.mean(expert_mask, axis=0)
    router_prob_per_expert = jnp.mean(routing_probs, axis=0)
    bl_loss = jnp.sum(tokens_per_expert * router_prob_per_expert[None, :]) * n_exp

    return (np.asarray(router_logits), np.asarray(top_i), np.asarray(top_w),
            np.asarray(bl_loss))


def _run_device(nc, in_maps, trace=False, **kw):
    return run_bass_kernel_spmd(nc, in_maps, core_ids=list(range(E)),
                                trace=trace, **kw)


def kernel(hidden_states, gate_w, gw, pw, ow, top_k, _trace=False, _res_out=None):
    hidden_states = np.asarray(hidden_states, dtype=np.float32)
    gate_w = np.asarray(gate_w, dtype=np.float32)
    gw = np.ascontiguousarray(np.asarray(gw, dtype=np.float32))
    pw = np.ascontiguousarray(np.asarray(pw, dtype=np.float32))
    ow = np.ascontiguousarray(np.asarray(ow, dtype=np.float32))
    k = int(top_k)

    B, S, _ = hidden_states.shape
    x = hidden_states.reshape(-1, D)
    T = x.shape[0]

    w_np = mybir.dt.np(W_DT)
    if gw.dtype != w_np:
        gw, pw, ow = (a.astype(w_np) for a in (gw, pw, ow))

    router_logits, top_i, top_w, bl_loss = _route(x, gate_w, k)

    # --- host dispatch (the "all-to-all") ---
    counts = np.bincount(top_i.ravel(), minlength=E)
    C = max(128, int(math.ceil(counts.max() / 128)) * 128)
    prog = _get_program(C)

    xT = np.ascontiguousarray(x.T)  # [D, T] so per-expert gathers are column slices
    idx_list, w_list, in_maps = [], [], []
    for e in range(E):
        sel = top_i == e                       # [T, k]
        idx = np.nonzero(sel.any(axis=1))[0]   # tokens routed to e
        wgt = (top_w[idx] * sel[idx]).sum(axis=1).astype(np.float32)
        xt = np.zeros((D, C), np.float32)
        xt[:, :idx.size] = xT[:, idx]
        idx_list.append(idx)
        w_list.append(wgt)
        in_maps.append({"xt": xt, "gw": gw[e], "pw": pw[e], "ow": ow[e]})

    res = _run_device(prog, in_maps, trace=_trace)
    if _res_out is not None:
        _res_out.append(res)

    final = np.zeros((T, D), np.float32)
    for e in range(E):
        idx = idx_list[e]
        ye = res.results[e]["yt"][:, :idx.size].T  # [n_e, D]
        final[idx] += ye * w_list[e][:, None]

    return (final.reshape(B, S, D),
            router_logits,
            np.float32(bl_loss))


# revision 33
# speedup vs baseline: 1.2108x; 1.0086x over previous
"""MoE (top-2 of 8 experts, SwiGLU) Trainium2 kernel — expert-parallel over 8 NeuronCores.

Strategy
--------
- Host computes the tiny router (T x D @ D x 8 = 0.03% of total FLOPs) with the
  exact same jax ops as the reference, so top-k selection matches bitwise.
- Token dispatch ("all-to-all") happens on the host: tokens routed to expert e
  are gathered (transposed, capacity-padded) and shipped to core e.
- Each of the 8 cores runs an identical SPMD Bass program: the SwiGLU expert
  FFN for its expert over its capacity-C token slab.  Matmuls use float32r
  (full fp32 data; streams at 1 cycle/row for moving dim >= 256, i.e. bf16
  speed with fp32-grade precision).
- Host scatter-adds the two expert outputs per token back together with the
  routing weights (exactly the reference's dense-combine math restricted to
  the nonzero entries).

Per-core device work: ~C x (3 matmuls of [*,1024]x[1024,1024]) ≈ 2.6e10 FLOPs
(sparse: 4x less than the reference's dense form), vs 1.03e11 dense.
"""

import math
from contextlib import ExitStack

import numpy as np

import concourse.bass as bass
import concourse.tile as tile
from concourse import bacc, mybir
from concourse.bass_utils import run_bass_kernel_spmd

D = 1024       # model dim
H = 1024       # per-expert hidden dim
E = 8          # experts == cores
TG = 512       # tokens per tile chunk (full PSUM bank; float32r full rate >= 256)
MM_DT = mybir.dt.float32r   # moving operand (activations)
W_DT = mybir.dt.float32r    # stationary operand (weights; must match moving dtype)
ACC_DT = mybir.dt.float32
# HW has a Silu LUT (one ACT op, one DVE mul); CoreSim only implements
# Sigmoid (one ACT op, two DVE muls).  Tests flip this off to simulate.
USE_SILU = True

_BUILD_CACHE: dict[int, object] = {}


def _chunks(C):
    """Split C tokens into chunks of 512 plus one ragged 128-multiple tail."""
    out = [TG] * (C // TG)
    if C % TG:
        out.append(C % TG)
    return out


def _build(C):
    """Build + compile the single-expert SwiGLU FFN program for capacity C.

    Computes yt = (silu(xt.T @ pw) * (xt.T @ gw)) @ ow, transposed:
    everything is laid out [feature, token] so no on-device transposes are
    needed (host ships x pre-transposed and un-transposes y).
    """
    assert C % 128 == 0
    nc = bacc.Bacc("TRN2", target_bir_lowering=False, debug=False, num_devices=E)
    xt = nc.dram_tensor("xt", [D, C], MM_DT, kind="ExternalInput").ap()
    gw = nc.dram_tensor("gw", [D, H], W_DT, kind="ExternalInput").ap()
    pw = nc.dram_tensor("pw", [D, H], W_DT, kind="ExternalInput").ap()
    ow = nc.dram_tensor("ow", [H, D], W_DT, kind="ExternalInput").ap()
    yt = nc.dram_tensor("yt", [D, C], ACC_DT, kind="ExternalOutput").ap()

    KB = D // 128  # contraction blocks for the first matmuls
    HB = H // 128  # hidden blocks

    # Partition-blocked 3D views: [(a p) m] -> [p, a, m] so each weight
    # matrix / token chunk moves as ONE large DMA (HWDGE trigger is ~600ns
    # of engine time each; batching to >=1MiB is the documented rule).
    xt_r = xt.rearrange("(a p) c -> p a c", p=128)
    yt_r = yt.rearrange("(a p) c -> p a c", p=128)
    gw_r = gw.rearrange("(a p) m -> p a m", p=128)
    pw_r = pw.rearrange("(a p) m -> p a m", p=128)
    ow_r = ow.rearrange("(a p) m -> p a m", p=128)

    with tile.TileContext(nc) as tc, ExitStack() as ctx:
        wpool = ctx.enter_context(tc.tile_pool(name="w", bufs=1))
        xpool = ctx.enter_context(tc.tile_pool(name="x", bufs=2))
        hpool = ctx.enter_context(tc.tile_pool(name="h", bufs=2))
        spool = ctx.enter_context(tc.tile_pool(name="s", bufs=2))
        ypool = ctx.enter_context(tc.tile_pool(name="y", bufs=1))
        gpsum = ctx.enter_context(tc.tile_pool(name="pg", bufs=3, space="PSUM"))
        ppsum = ctx.enter_context(tc.tile_pool(name="pp", bufs=3, space="PSUM"))
        ypsum = ctx.enter_context(tc.tile_pool(name="py", bufs=2, space="PSUM"))

        def load_x(col, tg):
            t = xpool.tile([128, KB, tg], MM_DT, tag="x")
            nc.sync.dma_start(t[:], xt_r[:, :, col:col + tg])
            return t

        def load_w(ap_r, prefix):
            # two halves: G can start after the first 4 k-blocks land, and
            # the trigger pipeline (one HWDGE ring) interleaves better.
            t = wpool.tile([128, KB, H], W_DT, tag=prefix)
            half = KB // 2
            nc.sync.dma_start(t[:, :half, :], ap_r[:, :half, :])
            nc.sync.dma_start(t[:, half:, :], ap_r[:, half:, :])
            return t

        # Issue order matters: PE starts on chunk 0's G matmuls as soon as
        # x(chunk0) + gw arrive; pw is needed ~14us later, ow ~28us later
        # (and the Y phase runs one chunk behind to hide ow's wire time).
        # x0/gw/pw are split per k-tile, interleaved in exactly the order
        # chunk 0's k-outer waves consume them, so PE unblocks tile-by-tile
        # at wire rate instead of waiting for whole matrices.
        chunk_list = _chunks(C)
        tg0 = chunk_list[0]
        xs_next = xpool.tile([128, KB, tg0], MM_DT, tag="x")
        gw_t = wpool.tile([128, KB, H], W_DT, tag="gw")
        pw_t = wpool.tile([128, KB, H], W_DT, tag="pw")
        for k in range(KB):
            nc.sync.dma_start(xs_next[:, k, :], xt_r[:, k, 0:tg0])
            nc.sync.dma_start(gw_t[:, k, :], gw_r[:, k, :])
        for k in range(KB):
            nc.sync.dma_start(pw_t[:, k, :], pw_r[:, k, :])
        ow_t = load_w(ow_r, "ow")

        pending_y = None  # (hs, col, tg) of the previous chunk

        def silu_mul(pp_t, g_t, tg, h):
            """ht = g * silu(p).  g_t may be PSUM or SBUF; pp_t is PSUM —
            every DVE op reads at most one PSUM operand."""
            if USE_SILU:
                sg = spool.tile([128, tg], ACC_DT, tag="sig", name="sg")
                nc.scalar.activation(
                    sg[:], pp_t[:], mybir.ActivationFunctionType.Silu)
                ht = hpool.tile([128, tg], MM_DT, tag=f"h{h}", name="ht")
                nc.vector.tensor_mul(ht[:], g_t[:], sg[:])
            else:
                sg = spool.tile([128, tg], ACC_DT, tag="sig", name="sg")
                nc.scalar.activation(
                    sg[:], pp_t[:], mybir.ActivationFunctionType.Sigmoid)
                sl = spool.tile([128, tg], ACC_DT, tag="sil", name="sl")
                nc.vector.tensor_mul(sl[:], pp_t[:], sg[:])
                ht = hpool.tile([128, tg], MM_DT, tag=f"h{h}", name="ht")
                nc.vector.tensor_mul(ht[:], g_t[:], sl[:])
            return ht

        def emit_y(hs, ycol, tg):
            ybig = ypool.tile([128, KB, tg], ACC_DT, tag="y")
            for dblk in range(KB):
                py = ypsum.tile([128, tg], ACC_DT, tag="py")
                for h in range(HB):
                    nc.tensor.matmul(
                        py[:], ow_t[:, h, dblk * 128:(dblk + 1) * 128], hs[h][:],
                        start=(h == 0), stop=(h == HB - 1))
                nc.vector.tensor_copy(ybig[:, dblk, :], py[:])
            # ACT's HWDGE ring: keep SyncE's ring free for input feeding
            nc.scalar.dma_start(yt_r[:, :, ycol:ycol + tg], ybig[:])

        def emit_gp_chunk0(xs, tg):
            """Chunk 0 with k-OUTER matmul order: h-outer needs the whole
            weight matrix within ~2us, but the wire delivers it over ~17us;
            k-outer consumes each 512KB k-tile right as it lands.  G results
            are copied out of PSUM to SBUF so P can reuse the banks (silu
            needs P of the same h before G could otherwise drain)."""
            hs = []
            g_sb = []
            # 4 concurrent accumulators per wave = 2 slots from the G/P pool
            # + 2 borrowed from the (not-yet-used) Y pool: stays within the
            # 8 PSUM banks without growing any pool.
            def wave_tiles(pool, tag):
                return [pool.tile([128, tg], ACC_DT, tag=tag, name=f"{tag}w0"),
                        pool.tile([128, tg], ACC_DT, tag=tag, name=f"{tag}w1"),
                        ypsum.tile([128, tg], ACC_DT, tag="py", name=f"{tag}w2"),
                        ypsum.tile([128, tg], ACC_DT, tag="py", name=f"{tag}w3")]
            for wave in range(2):          # h-blocks 0..3, then 4..7
                pgs = wave_tiles(gpsum, "pg")
                for k in range(KB):
                    for i in range(4):
                        h = wave * 4 + i
                        nc.tensor.matmul(
                            pgs[i][:], gw_t[:, k, h * 128:(h + 1) * 128],
                            xs[:, k, :], start=(k == 0), stop=(k == KB - 1))
                for i in range(4):
                    g = wpool.tile([128, tg], ACC_DT, tag=f"gsb{wave * 4 + i}")
                    nc.vector.tensor_copy(g[:], pgs[i][:])
                    g_sb.append(g)
            for wave in range(2):
                pps = wave_tiles(ppsum, "pp")
                for k in range(KB):
                    for i in range(4):
                        h = wave * 4 + i
                        nc.tensor.matmul(
                            pps[i][:], pw_t[:, k, h * 128:(h + 1) * 128],
                            xs[:, k, :], start=(k == 0), stop=(k == KB - 1))
                for i in range(4):
                    h = wave * 4 + i
                    hs.append(silu_mul(pps[i], g_sb[h], tg, h))
            return hs

        def emit_gp(xs, tg):
            hs = []
            for h in range(HB):
                pg = gpsum.tile([128, tg], ACC_DT, tag="pg")
                for k in range(KB):
                    nc.tensor.matmul(
                        pg[:], gw_t[:, k, h * 128:(h + 1) * 128], xs[:, k, :],
                        start=(k == 0), stop=(k == KB - 1))
                pp = ppsum.tile([128, tg], ACC_DT, tag="pp")
                for k in range(KB):
                    nc.tensor.matmul(
                        pp[:], pw_t[:, k, h * 128:(h + 1) * 128], xs[:, k, :],
                        start=(k == 0), stop=(k == KB - 1))
                hs.append(silu_mul(pp, pg, tg, h))
            return hs

        col = 0
        for ci, tg in enumerate(chunk_list):
            xs = xs_next
            if ci + 1 < len(chunk_list):
                xs_next = load_x(col + tg, chunk_list[ci + 1])

            hs = emit_gp_chunk0(xs, tg) if ci == 0 else emit_gp(xs, tg)

            if pending_y is not None:
                emit_y(*pending_y)
            pending_y = (hs, col, tg)
            col += tg
        emit_y(*pending_y)

    nc.compile()
    return nc


def _get_program(C):
    if C not in _BUILD_CACHE:
        _BUILD_CACHE[C] = _build(C)
    return _BUILD_CACHE[C]


def _route(x, gate_w, k):
    """Router with the reference's exact jax ops (bitwise-matching top-k)."""
    import jax
    import jax.numpy as jnp

    router_logits = jnp.asarray(x) @ jnp.asarray(gate_w)
    routing_probs = jax.nn.softmax(router_logits.astype(jnp.float32), axis=-1)
    top_w, top_i = jax.lax.top_k(routing_probs, k)
    top_w = top_w / jnp.sum(top_w, axis=-1, keepdims=True)
    top_w = top_w.astype(jnp.float32)

    n_exp = gate_w.shape[1]
    expert_mask = jax.nn.one_hot(top_i, n_exp, dtype=jnp.float32)
    tokens_per_expert = jnp.mean(expert_mask, axis=0)
    router_prob_per_expert = jnp.mean(routing_probs, axis=0)
    bl_loss = jnp.sum(tokens_per_expert * router_prob_per_expert[None, :]) * n_exp

    return (np.asarray(router_logits), np.asarray(top_i), np.asarray(top_w),
            np.asarray(bl_loss))


def _run_device(nc, in_maps, trace=False, **kw):
    return run_bass_kernel_spmd(nc, in_maps, core_ids=list(range(E)),
                                trace=trace, **kw)


def kernel(hidden_states, gate_w, gw, pw, ow, top_k, _trace=False, _res_out=None):
    hidden_states = np.asarray(hidden_states, dtype=np.float32)
    gate_w = np.asarray(gate_w, dtype=np.float32)
    gw = np.ascontiguousarray(np.asarray(gw, dtype=np.float32))
    pw = np.ascontiguousarray(np.asarray(pw, dtype=np.float32))
    ow = np.ascontiguousarray(np.asarray(ow, dtype=np.float32))
    k = int(top_k)

    B, S, _ = hidden_states.shape
    x = hidden_states.reshape(-1, D)
    T = x.shape[0]

    w_np = mybir.dt.np(W_DT)
    if gw.dtype != w_np:
        gw, pw, ow = (a.astype(w_np) for a in (gw, pw, ow))

    router_logits, top_i, top_w, bl_loss = _route(x, gate_w, k)

    # --- host dispatch (the "all-to-all") ---
    counts = np.bincount(top_i.ravel(), minlength=E)
    C = max(128, int(math.ceil(counts.max() / 128)) * 128)
    prog = _get_program(C)

    xT = np.ascontiguousarray(x.T)  # [D, T] so per-expert gathers are column slices
    idx_list, w_list, in_maps = [], [], []
    for e in range(E):
        sel = top_i == e                       # [T, k]
        idx = np.nonzero(sel.any(axis=1))[0]   # tokens routed to e
        wgt = (top_w[idx] * sel[idx]).sum(axis=1).astype(np.float32)
        xt = np.zeros((D, C), np.float32)
        xt[:, :idx.size] = xT[:, idx]
        idx_list.append(idx)
        w_list.append(wgt)
        in_maps.append({"xt": xt, "gw": gw[e], "pw": pw[e], "ow": ow[e]})

    res = _run_device(prog, in_maps, trace=_trace)
    if _res_out is not None:
        _res_out.append(res)

    final = np.zeros((T, D), np.float32)
    for e in range(E):
        idx = idx_list[e]
        ye = res.results[e]["yt"][:, :idx.size].T  # [n_e, D]
        final[idx] += ye * w_list[e][:, None]

    return (final.reshape(B, S, D),
            router_logits,
            np.float32(bl_loss))


# revision 34
# speedup vs baseline: 1.2222x; 1.0094x over previous
"""MoE (top-2 of 8 experts, SwiGLU) Trainium2 kernel — expert-parallel over 8 NeuronCores.

Strategy
--------
- Host computes the tiny router (T x D @ D x 8 = 0.03% of total FLOPs) with the
  exact same jax ops as the reference, so top-k selection matches bitwise.
- Token dispatch ("all-to-all") happens on the host: tokens routed to expert e
  are gathered (transposed, capacity-padded) and shipped to core e.
- Each of the 8 cores runs an identical SPMD Bass program: the SwiGLU expert
  FFN for its expert over its capacity-C token slab.  Matmuls use float32r
  (full fp32 data; streams at 1 cycle/row for moving dim >= 256, i.e. bf16
  speed with fp32-grade precision).
- Host scatter-adds the two expert outputs per token back together with the
  routing weights (exactly the reference's dense-combine math restricted to
  the nonzero entries).

Per-core device work: ~C x (3 matmuls of [*,1024]x[1024,1024]) ≈ 2.6e10 FLOPs
(sparse: 4x less than the reference's dense form), vs 1.03e11 dense.
"""

import math
from contextlib import ExitStack

import numpy as np

import concourse.bass as bass
import concourse.tile as tile
from concourse import bacc, mybir
from concourse.bass_utils import run_bass_kernel_spmd

D = 1024       # model dim
H = 1024       # per-expert hidden dim
E = 8          # experts == cores
TG = 512       # tokens per tile chunk (full PSUM bank; float32r full rate >= 256)
MM_DT = mybir.dt.float32r   # moving operand (activations)
W_DT = mybir.dt.float32r    # stationary operand (weights; must match moving dtype)
ACC_DT = mybir.dt.float32
# HW has a Silu LUT (one ACT op, one DVE mul); CoreSim only implements
# Sigmoid (one ACT op, two DVE muls).  Tests flip this off to simulate.
USE_SILU = True

_BUILD_CACHE: dict[int, object] = {}


def _chunks(C):
    """Split C tokens into chunks of 512 plus one ragged 128-multiple tail."""
    out = [TG] * (C // TG)
    if C % TG:
        out.append(C % TG)
    return out


def _build(C):
    """Build + compile the single-expert SwiGLU FFN program for capacity C.

    Computes yt = (silu(xt.T @ pw) * (xt.T @ gw)) @ ow, transposed:
    everything is laid out [feature, token] so no on-device transposes are
    needed (host ships x pre-transposed and un-transposes y).
    """
    assert C % 128 == 0
    nc = bacc.Bacc("TRN2", target_bir_lowering=False, debug=False, num_devices=E)
    xt = nc.dram_tensor("xt", [D, C], MM_DT, kind="ExternalInput").ap()
    gw = nc.dram_tensor("gw", [D, H], W_DT, kind="ExternalInput").ap()
    pw = nc.dram_tensor("pw", [D, H], W_DT, kind="ExternalInput").ap()
    ow = nc.dram_tensor("ow", [H, D], W_DT, kind="ExternalInput").ap()
    yt = nc.dram_tensor("yt", [D, C], ACC_DT, kind="ExternalOutput").ap()

    KB = D // 128  # contraction blocks for the first matmuls
    HB = H // 128  # hidden blocks

    # Partition-blocked 3D views: [(a p) m] -> [p, a, m] so each weight
    # matrix / token chunk moves as ONE large DMA (HWDGE trigger is ~600ns
    # of engine time each; batching to >=1MiB is the documented rule).
    xt_r = xt.rearrange("(a p) c -> p a c", p=128)
    yt_r = yt.rearrange("(a p) c -> p a c", p=128)
    gw_r = gw.rearrange("(a p) m -> p a m", p=128)
    pw_r = pw.rearrange("(a p) m -> p a m", p=128)
    ow_r = ow.rearrange("(a p) m -> p a m", p=128)

    with tile.TileContext(nc) as tc, ExitStack() as ctx:
        wpool = ctx.enter_context(tc.tile_pool(name="w", bufs=1))
        xpool = ctx.enter_context(tc.tile_pool(name="x", bufs=2))
        hpool = ctx.enter_context(tc.tile_pool(name="h", bufs=2))
        spool = ctx.enter_context(tc.tile_pool(name="s", bufs=2))
        ypool = ctx.enter_context(tc.tile_pool(name="y", bufs=1))
        gpsum = ctx.enter_context(tc.tile_pool(name="pg", bufs=3, space="PSUM"))
        ppsum = ctx.enter_context(tc.tile_pool(name="pp", bufs=3, space="PSUM"))
        ypsum = ctx.enter_context(tc.tile_pool(name="py", bufs=2, space="PSUM"))

        def load_x(col, tg):
            t = xpool.tile([128, KB, tg], MM_DT, tag="x")
            nc.sync.dma_start(t[:], xt_r[:, :, col:col + tg])
            return t

        def load_w(ap_r, prefix):
            # two halves: G can start after the first 4 k-blocks land, and
            # the trigger pipeline (one HWDGE ring) interleaves better.
            t = wpool.tile([128, KB, H], W_DT, tag=prefix)
            half = KB // 2
            nc.sync.dma_start(t[:, :half, :], ap_r[:, :half, :])
            nc.sync.dma_start(t[:, half:, :], ap_r[:, half:, :])
            return t

        # Issue order matters: PE starts on chunk 0's G matmuls as soon as
        # x(chunk0) + gw arrive; pw is needed ~14us later, ow ~28us later
        # (and the Y phase runs one chunk behind to hide ow's wire time).
        # x0/gw/pw are split per k-tile, interleaved in exactly the order
        # chunk 0's k-outer waves consume them, so PE unblocks tile-by-tile
        # at wire rate instead of waiting for whole matrices.
        chunk_list = _chunks(C)
        tg0 = chunk_list[0]
        xs_next = xpool.tile([128, KB, tg0], MM_DT, tag="x")
        gw_t = wpool.tile([128, KB, H], W_DT, tag="gw")
        pw_t = wpool.tile([128, KB, H], W_DT, tag="pw")
        for k in range(KB):
            nc.sync.dma_start(xs_next[:, k, :], xt_r[:, k, 0:tg0])
            nc.sync.dma_start(gw_t[:, k, :], gw_r[:, k, :])
        for k in range(KB):
            nc.sync.dma_start(pw_t[:, k, :], pw_r[:, k, :])
        ow_t = load_w(ow_r, "ow")

        pending_y = None  # (hs, col, tg) of the previous chunk

        def silu_mul(pp_t, g_t, tg, h):
            """ht = g * silu(p).  g_t may be PSUM or SBUF; pp_t is PSUM —
            every DVE op reads at most one PSUM operand."""
            if USE_SILU:
                sg = spool.tile([128, tg], ACC_DT, tag="sig", name="sg")
                nc.scalar.activation(
                    sg[:], pp_t[:], mybir.ActivationFunctionType.Silu)
                ht = hpool.tile([128, tg], MM_DT, tag=f"h{h}", name="ht")
                nc.vector.tensor_mul(ht[:], g_t[:], sg[:])
            else:
                sg = spool.tile([128, tg], ACC_DT, tag="sig", name="sg")
                nc.scalar.activation(
                    sg[:], pp_t[:], mybir.ActivationFunctionType.Sigmoid)
                sl = spool.tile([128, tg], ACC_DT, tag="sil", name="sl")
                nc.vector.tensor_mul(sl[:], pp_t[:], sg[:])
                ht = hpool.tile([128, tg], MM_DT, tag=f"h{h}", name="ht")
                nc.vector.tensor_mul(ht[:], g_t[:], sl[:])
            return ht

        def emit_y(hs, ycol, tg):
            # Two half-slabs with independent DMAs: the next chunk's copies
            # only wait on the matching HALF's DRAM write, and the first
            # half's DMA fires 4 d-blocks earlier (keeps PE warm at the tail).
            half = KB // 2
            ysb = [ypool.tile([128, half, tg], ACC_DT, tag="y0", name="y0"),
                   ypool.tile([128, half, tg], ACC_DT, tag="y1", name="y1")]
            for dblk in range(KB):
                py = ypsum.tile([128, tg], ACC_DT, tag="py")
                for h in range(HB):
                    nc.tensor.matmul(
                        py[:], ow_t[:, h, dblk * 128:(dblk + 1) * 128], hs[h][:],
                        start=(h == 0), stop=(h == HB - 1))
                nc.vector.tensor_copy(ysb[dblk // half][:, dblk % half, :], py[:])
                if dblk == half - 1:
                    # ACT's HWDGE ring: keep SyncE's ring free for inputs
                    nc.scalar.dma_start(
                        yt_r[:, :half, ycol:ycol + tg], ysb[0][:])
            nc.scalar.dma_start(yt_r[:, half:, ycol:ycol + tg], ysb[1][:])

        def emit_gp_chunk0(xs, tg):
            """Chunk 0 with k-OUTER matmul order: h-outer needs the whole
            weight matrix within ~2us, but the wire delivers it over ~17us;
            k-outer consumes each 512KB k-tile right as it lands.  G results
            are copied out of PSUM to SBUF so P can reuse the banks (silu
            needs P of the same h before G could otherwise drain)."""
            hs = []
            g_sb = []
            # 4 concurrent accumulators per wave = 2 slots from the G/P pool
            # + 2 borrowed from the (not-yet-used) Y pool: stays within the
            # 8 PSUM banks without growing any pool.
            def wave_tiles(pool, tag):
                return [pool.tile([128, tg], ACC_DT, tag=tag, name=f"{tag}w0"),
                        pool.tile([128, tg], ACC_DT, tag=tag, name=f"{tag}w1"),
                        ypsum.tile([128, tg], ACC_DT, tag="py", name=f"{tag}w2"),
                        ypsum.tile([128, tg], ACC_DT, tag="py", name=f"{tag}w3")]
            for wave in range(2):          # h-blocks 0..3, then 4..7
                pgs = wave_tiles(gpsum, "pg")
                for k in range(KB):
                    for i in range(4):
                        h = wave * 4 + i
                        nc.tensor.matmul(
                            pgs[i][:], gw_t[:, k, h * 128:(h + 1) * 128],
                            xs[:, k, :], start=(k == 0), stop=(k == KB - 1))
                for i in range(4):
                    g = wpool.tile([128, tg], ACC_DT, tag=f"gsb{wave * 4 + i}")
                    nc.vector.tensor_copy(g[:], pgs[i][:])
                    g_sb.append(g)
            for wave in range(2):
                pps = wave_tiles(ppsum, "pp")
                for k in range(KB):
                    for i in range(4):
                        h = wave * 4 + i
                        nc.tensor.matmul(
                            pps[i][:], pw_t[:, k, h * 128:(h + 1) * 128],
                            xs[:, k, :], start=(k == 0), stop=(k == KB - 1))
                for i in range(4):
                    h = wave * 4 + i
                    hs.append(silu_mul(pps[i], g_sb[h], tg, h))
            return hs

        def emit_gp(xs, tg):
            hs = []
            for h in range(HB):
                pg = gpsum.tile([128, tg], ACC_DT, tag="pg")
                for k in range(KB):
                    nc.tensor.matmul(
                        pg[:], gw_t[:, k, h * 128:(h + 1) * 128], xs[:, k, :],
                        start=(k == 0), stop=(k == KB - 1))
                pp = ppsum.tile([128, tg], ACC_DT, tag="pp")
                for k in range(KB):
                    nc.tensor.matmul(
                        pp[:], pw_t[:, k, h * 128:(h + 1) * 128], xs[:, k, :],
                        start=(k == 0), stop=(k == KB - 1))
                hs.append(silu_mul(pp, pg, tg, h))
            return hs

        col = 0
        for ci, tg in enumerate(chunk_list):
            xs = xs_next
            if ci + 1 < len(chunk_list):
                xs_next = load_x(col + tg, chunk_list[ci + 1])

            hs = emit_gp_chunk0(xs, tg) if ci == 0 else emit_gp(xs, tg)

            if pending_y is not None:
                emit_y(*pending_y)
            pending_y = (hs, col, tg)
            col += tg
        emit_y(*pending_y)

    nc.compile()
    return nc


def _get_program(C):
    if C not in _BUILD_CACHE:
        _BUILD_CACHE[C] = _build(C)
    return _BUILD_CACHE[C]


def _route(x, gate_w, k):
    """Router with the reference's exact jax ops (bitwise-matching top-k)."""
    import jax
    import jax.numpy as jnp

    router_logits = jnp.asarray(x) @ jnp.asarray(gate_w)
    routing_probs = jax.nn.softmax(router_logits.astype(jnp.float32), axis=-1)
    top_w, top_i = jax.lax.top_k(routing_probs, k)
    top_w = top_w / jnp.sum(top_w, axis=-1, keepdims=True)
    top_w = top_w.astype(jnp.float32)

    n_exp = gate_w.shape[1]
    expert_mask = jax.nn.one_hot(top_i, n_exp, dtype=jnp.float32)
    tokens_per_expert = jnp.mean(expert_mask, axis=0)
    router_prob_per_expert = jnp.mean(routing_probs, axis=0)
    bl_loss = jnp.sum(tokens_per_expert * router_prob_per_expert[None, :]) * n_exp

    return (np.asarray(router_logits), np.asarray(top_i), np.asarray(top_w),
            np.asarray(bl_loss))


def _run_device(nc, in_maps, trace=False, **kw):
    return run_bass_kernel_spmd(nc, in_maps, core_ids=list(range(E)),
                                trace=trace, **kw)


def kernel(hidden_states, gate_w, gw, pw, ow, top_k, _trace=False, _res_out=None):
    hidden_states = np.asarray(hidden_states, dtype=np.float32)
    gate_w = np.asarray(gate_w, dtype=np.float32)
    gw = np.ascontiguousarray(np.asarray(gw, dtype=np.float32))
    pw = np.ascontiguousarray(np.asarray(pw, dtype=np.float32))
    ow = np.ascontiguousarray(np.asarray(ow, dtype=np.float32))
    k = int(top_k)

    B, S, _ = hidden_states.shape
    x = hidden_states.reshape(-1, D)
    T = x.shape[0]

    w_np = mybir.dt.np(W_DT)
    if gw.dtype != w_np:
        gw, pw, ow = (a.astype(w_np) for a in (gw, pw, ow))

    router_logits, top_i, top_w, bl_loss = _route(x, gate_w, k)

    # --- host dispatch (the "all-to-all") ---
    counts = np.bincount(top_i.ravel(), minlength=E)
    C = max(128, int(math.ceil(counts.max() / 128)) * 128)
    prog = _get_program(C)

    xT = np.ascontiguousarray(x.T)  # [D, T] so per-expert gathers are column slices
    idx_list, w_list, in_maps = [], [], []
    for e in range(E):
        sel = top_i == e                       # [T, k]
        idx = np.nonzero(sel.any(axis=1))[0]   # tokens routed to e
        wgt = (top_w[idx] * sel[idx]).sum(axis=1).astype(np.float32)
        xt = np.zeros((D, C), np.float32)
        xt[:, :idx.size] = xT[:, idx]
        idx_list.append(idx)
        w_list.append(wgt)
        in_maps.append({"xt": xt, "gw": gw[e], "pw": pw[e], "ow": ow[e]})

    res = _run_device(prog, in_maps, trace=_trace)
    if _res_out is not None:
        _res_out.append(res)

    final = np.zeros((T, D), np.float32)
    for e in range(E):
        idx = idx_list[e]
        ye = res.results[e]["yt"][:, :idx.size].T  # [n_e, D]
        final[idx] += ye * w_list[e][:, None]

    return (final.reshape(B, S, D),
            router_logits,
            np.float32(bl_loss))


# revision 36
# speedup vs baseline: 1.2271x; 1.0040x over previous
"""MoE (top-2 of 8 experts, SwiGLU) Trainium2 kernel — expert-parallel over 8 NeuronCores.

Strategy
--------
- Host computes the tiny router (T x D @ D x 8 = 0.03% of total FLOPs) with the
  exact same jax ops as the reference, so top-k selection matches bitwise.
- Token dispatch ("all-to-all") happens on the host: tokens routed to expert e
  are gathered (transposed, capacity-padded) and shipped to core e.
- Each of the 8 cores runs an identical SPMD Bass program: the SwiGLU expert
  FFN for its expert over its capacity-C token slab.  Matmuls use float32r
  (full fp32 data; streams at 1 cycle/row for moving dim >= 256, i.e. bf16
  speed with fp32-grade precision).
- Host scatter-adds the two expert outputs per token back together with the
  routing weights (exactly the reference's dense-combine math restricted to
  the nonzero entries).

Per-core device work: ~C x (3 matmuls of [*,1024]x[1024,1024]) ≈ 2.6e10 FLOPs
(sparse: 4x less than the reference's dense form), vs 1.03e11 dense.
"""

import math
from contextlib import ExitStack

import numpy as np

import concourse.bass as bass
import concourse.tile as tile
from concourse import bacc, mybir
from concourse.bass_utils import run_bass_kernel_spmd

D = 1024       # model dim
H = 1024       # per-expert hidden dim
E = 8          # experts == cores
TG = 512       # tokens per tile chunk (full PSUM bank; float32r full rate >= 256)
MM_DT = mybir.dt.float32r   # moving operand (activations)
W_DT = mybir.dt.float32r    # stationary operand (weights; must match moving dtype)
ACC_DT = mybir.dt.float32
# HW has a Silu LUT (one ACT op, one DVE mul); CoreSim only implements
# Sigmoid (one ACT op, two DVE muls).  Tests flip this off to simulate.
USE_SILU = True

_BUILD_CACHE: dict[int, object] = {}


def _chunks(C):
    """Split C tokens into chunks of 512 plus one ragged 128-multiple tail."""
    out = [TG] * (C // TG)
    if C % TG:
        out.append(C % TG)
    return out


def _build(C):
    """Build + compile the single-expert SwiGLU FFN program for capacity C.

    Computes yt = (silu(xt.T @ pw) * (xt.T @ gw)) @ ow, transposed:
    everything is laid out [feature, token] so no on-device transposes are
    needed (host ships x pre-transposed and un-transposes y).
    """
    assert C % 128 == 0
    nc = bacc.Bacc("TRN2", target_bir_lowering=False, debug=False, num_devices=E)
    xt = nc.dram_tensor("xt", [D, C], MM_DT, kind="ExternalInput").ap()
    gw = nc.dram_tensor("gw", [D, H], W_DT, kind="ExternalInput").ap()
    pw = nc.dram_tensor("pw", [D, H], W_DT, kind="ExternalInput").ap()
    ow = nc.dram_tensor("ow", [H, D], W_DT, kind="ExternalInput").ap()
    yt = nc.dram_tensor("yt", [D, C], ACC_DT, kind="ExternalOutput").ap()

    KB = D // 128  # contraction blocks for the first matmuls
    HB = H // 128  # hidden blocks

    # Partition-blocked 3D views: [(a p) m] -> [p, a, m] so each weight
    # matrix / token chunk moves as ONE large DMA (HWDGE trigger is ~600ns
    # of engine time each; batching to >=1MiB is the documented rule).
    xt_r = xt.rearrange("(a p) c -> p a c", p=128)
    yt_r = yt.rearrange("(a p) c -> p a c", p=128)
    gw_r = gw.rearrange("(a p) m -> p a m", p=128)
    pw_r = pw.rearrange("(a p) m -> p a m", p=128)
    ow_r = ow.rearrange("(a p) m -> p a m", p=128)

    with tile.TileContext(nc) as tc, ExitStack() as ctx:
        wpool = ctx.enter_context(tc.tile_pool(name="w", bufs=1))
        xpool = ctx.enter_context(tc.tile_pool(name="x", bufs=2))
        hpool = ctx.enter_context(tc.tile_pool(name="h", bufs=2))
        spool = ctx.enter_context(tc.tile_pool(name="s", bufs=2))
        ypool = ctx.enter_context(tc.tile_pool(name="y", bufs=1))
        gpsum = ctx.enter_context(tc.tile_pool(name="pg", bufs=3, space="PSUM"))
        ppsum = ctx.enter_context(tc.tile_pool(name="pp", bufs=3, space="PSUM"))
        ypsum = ctx.enter_context(tc.tile_pool(name="py", bufs=2, space="PSUM"))

        def load_x(col, tg):
            t = xpool.tile([128, KB, tg], MM_DT, tag="x")
            nc.sync.dma_start(t[:], xt_r[:, :, col:col + tg])
            return t

        def load_w(ap_r, prefix):
            # two halves: G can start after the first 4 k-blocks land, and
            # the trigger pipeline (one HWDGE ring) interleaves better.
            t = wpool.tile([128, KB, H], W_DT, tag=prefix)
            half = KB // 2
            nc.sync.dma_start(t[:, :half, :], ap_r[:, :half, :])
            nc.sync.dma_start(t[:, half:, :], ap_r[:, half:, :])
            return t

        # Issue order matters: PE starts on chunk 0's G matmuls as soon as
        # x(chunk0) + gw arrive; pw is needed ~14us later, ow ~28us later
        # (and the Y phase runs one chunk behind to hide ow's wire time).
        # x0/gw/pw are split per k-tile, interleaved in exactly the order
        # chunk 0's k-outer waves consume them, so PE unblocks tile-by-tile
        # at wire rate instead of waiting for whole matrices.
        chunk_list = _chunks(C)
        tg0 = chunk_list[0]
        xs_next = xpool.tile([128, KB, tg0], MM_DT, tag="x")
        gw_t = wpool.tile([128, KB, H], W_DT, tag="gw")
        pw_t = wpool.tile([128, KB, H], W_DT, tag="pw")
        for k in range(KB):
            # x0 triggers on the (startup-idle) ACT ring, weights on SyncE's:
            # the two trigger pipelines run in parallel instead of 16 deep.
            nc.scalar.dma_start(xs_next[:, k, :], xt_r[:, k, 0:tg0])
            nc.sync.dma_start(gw_t[:, k, :], gw_r[:, k, :])
        for k in range(KB):
            nc.sync.dma_start(pw_t[:, k, :], pw_r[:, k, :])
        ow_t = load_w(ow_r, "ow")

        pending_y = None  # (hs, col, tg) of the previous chunk

        def silu_mul(pp_t, g_t, tg, h):
            """ht = g * silu(p).  g_t may be PSUM or SBUF; pp_t is PSUM —
            every DVE op reads at most one PSUM operand."""
            if USE_SILU:
                sg = spool.tile([128, tg], ACC_DT, tag="sig", name="sg")
                nc.scalar.activation(
                    sg[:], pp_t[:], mybir.ActivationFunctionType.Silu)
                ht = hpool.tile([128, tg], MM_DT, tag=f"h{h}", name="ht")
                nc.vector.tensor_mul(ht[:], g_t[:], sg[:])
            else:
                sg = spool.tile([128, tg], ACC_DT, tag="sig", name="sg")
                nc.scalar.activation(
                    sg[:], pp_t[:], mybir.ActivationFunctionType.Sigmoid)
                sl = spool.tile([128, tg], ACC_DT, tag="sil", name="sl")
                nc.vector.tensor_mul(sl[:], pp_t[:], sg[:])
                ht = hpool.tile([128, tg], MM_DT, tag=f"h{h}", name="ht")
                nc.vector.tensor_mul(ht[:], g_t[:], sl[:])
            return ht

        def emit_y(hs, ycol, tg):
            # Two half-slabs with independent DMAs: the next chunk's copies
            # only wait on the matching HALF's DRAM write, and the first
            # half's DMA fires 4 d-blocks earlier (keeps PE warm at the tail).
            half = KB // 2
            ysb = [ypool.tile([128, half, tg], ACC_DT, tag="y0", name="y0"),
                   ypool.tile([128, half, tg], ACC_DT, tag="y1", name="y1")]
            for dblk in range(KB):
                py = ypsum.tile([128, tg], ACC_DT, tag="py")
                for h in range(HB):
                    nc.tensor.matmul(
                        py[:], ow_t[:, h, dblk * 128:(dblk + 1) * 128], hs[h][:],
                        start=(h == 0), stop=(h == HB - 1))
                nc.vector.tensor_copy(ysb[dblk // half][:, dblk % half, :], py[:])
                if dblk == half - 1:
                    # ACT's HWDGE ring: keep SyncE's ring free for inputs
                    nc.scalar.dma_start(
                        yt_r[:, :half, ycol:ycol + tg], ysb[0][:])
            nc.scalar.dma_start(yt_r[:, half:, ycol:ycol + tg], ysb[1][:])

        def emit_gp_chunk0(xs, tg):
            """Chunk 0 with k-OUTER matmul order: h-outer needs the whole
            weight matrix within ~2us, but the wire delivers it over ~17us.
            All 8 h-accumulators live at once (3 pg + 3 pp + 2 py banks =
            the full 8 PSUM banks), so each k-step consumes (x0[k], gw[k])
            right as it lands — no second pass stuck behind wire waits in
            PE program order.  G results are copied to SBUF so the P phase
            can reuse the banks (silu needs P of the same h before G could
            otherwise drain)."""
            hs = []

            def bank8(prefix):
                return (
                    [gpsum.tile([128, tg], ACC_DT, tag="pg", name=f"{prefix}a{i}")
                     for i in range(3)]
                    + [ppsum.tile([128, tg], ACC_DT, tag="pp", name=f"{prefix}b{i}")
                       for i in range(3)]
                    + [ypsum.tile([128, tg], ACC_DT, tag="py", name=f"{prefix}c{i}")
                       for i in range(2)])

            pgs = bank8("g")
            for k in range(KB):
                for h in range(HB):
                    nc.tensor.matmul(
                        pgs[h][:], gw_t[:, k, h * 128:(h + 1) * 128],
                        xs[:, k, :], start=(k == 0), stop=(k == KB - 1))
            g_sb = []
            for h in range(HB):
                g = wpool.tile([128, tg], ACC_DT, tag=f"gsb{h}", name="g")
                nc.vector.tensor_copy(g[:], pgs[h][:])
                g_sb.append(g)
            pps = bank8("p")
            for k in range(KB):
                for h in range(HB):
                    nc.tensor.matmul(
                        pps[h][:], pw_t[:, k, h * 128:(h + 1) * 128],
                        xs[:, k, :], start=(k == 0), stop=(k == KB - 1))
            for h in range(HB):
                hs.append(silu_mul(pps[h], g_sb[h], tg, h))
            return hs

        def emit_gp(xs, tg):
            hs = []
            for h in range(HB):
                pg = gpsum.tile([128, tg], ACC_DT, tag="pg")
                for k in range(KB):
                    nc.tensor.matmul(
                        pg[:], gw_t[:, k, h * 128:(h + 1) * 128], xs[:, k, :],
                        start=(k == 0), stop=(k == KB - 1))
                pp = ppsum.tile([128, tg], ACC_DT, tag="pp")
                for k in range(KB):
                    nc.tensor.matmul(
                        pp[:], pw_t[:, k, h * 128:(h + 1) * 128], xs[:, k, :],
                        start=(k == 0), stop=(k == KB - 1))
                hs.append(silu_mul(pp, pg, tg, h))
            return hs

        col = 0
        for ci, tg in enumerate(chunk_list):
            xs = xs_next
            if ci + 1 < len(chunk_list):
                xs_next = load_x(col + tg, chunk_list[ci + 1])

            hs = emit_gp_chunk0(xs, tg) if ci == 0 else emit_gp(xs, tg)

            if pending_y is not None:
                emit_y(*pending_y)
            pending_y = (hs, col, tg)
            col += tg
        emit_y(*pending_y)

    nc.compile()
    return nc


def _get_program(C):
    if C not in _BUILD_CACHE:
        _BUILD_CACHE[C] = _build(C)
    return _BUILD_CACHE[C]


def _route(x, gate_w, k):
    """Router with the reference's exact jax ops (bitwise-matching top-k)."""
    import jax
    import jax.numpy as jnp

    router_logits = jnp.asarray(x) @ jnp.asarray(gate_w)
    routing_probs = jax.nn.softmax(router_logits.astype(jnp.float32), axis=-1)
    top_w, top_i = jax.lax.top_k(routing_probs, k)
    top_w = top_w / jnp.sum(top_w, axis=-1, keepdims=True)
    top_w = top_w.astype(jnp.float32)

    n_exp = gate_w.shape[1]
    expert_mask = jax.nn.one_hot(top_i, n_exp, dtype=jnp.float32)
    tokens_per_expert = jnp.mean(expert_mask, axis=0)
    router_prob_per_expert = jnp.mean(routing_probs, axis=0)
    bl_loss = jnp.sum(tokens_per_expert * router_prob_per_expert[None, :]) * n_exp

    return (np.asarray(router_logits), np.asarray(top_i), np.asarray(top_w),
            np.asarray(bl_loss))


def _run_device(nc, in_maps, trace=False, **kw):
    return run_bass_kernel_spmd(nc, in_maps, core_ids=list(range(E)),
                                trace=trace, **kw)


def kernel(hidden_states, gate_w, gw, pw, ow, top_k, _trace=False, _res_out=None):
    hidden_states = np.asarray(hidden_states, dtype=np.float32)
    gate_w = np.asarray(gate_w, dtype=np.float32)
    gw = np.ascontiguousarray(np.asarray(gw, dtype=np.float32))
    pw = np.ascontiguousarray(np.asarray(pw, dtype=np.float32))
    ow = np.ascontiguousarray(np.asarray(ow, dtype=np.float32))
    k = int(top_k)

    B, S, _ = hidden_states.shape
    x = hidden_states.reshape(-1, D)
    T = x.shape[0]

    w_np = mybir.dt.np(W_DT)
    if gw.dtype != w_np:
        gw, pw, ow = (a.astype(w_np) for a in (gw, pw, ow))

    router_logits, top_i, top_w, bl_loss = _route(x, gate_w, k)

    # --- host dispatch (the "all-to-all") ---
    counts = np.bincount(top_i.ravel(), minlength=E)
    C = max(128, int(math.ceil(counts.max() / 128)) * 128)
    prog = _get_program(C)

    xT = np.ascontiguousarray(x.T)  # [D, T] so per-expert gathers are column slices
    idx_list, w_list, in_maps = [], [], []
    for e in range(E):
        sel = top_i == e                       # [T, k]
        idx = np.nonzero(sel.any(axis=1))[0]   # tokens routed to e
        wgt = (top_w[idx] * sel[idx]).sum(axis=1).astype(np.float32)
        xt = np.zeros((D, C), np.float32)
        xt[:, :idx.size] = xT[:, idx]
        idx_list.append(idx)
        w_list.append(wgt)
        in_maps.append({"xt": xt, "gw": gw[e], "pw": pw[e], "ow": ow[e]})

    res = _run_device(prog, in_maps, trace=_trace)
    if _res_out is not None:
        _res_out.append(res)

    final = np.zeros((T, D), np.float32)
    for e in range(E):
        idx = idx_list[e]
        ye = res.results[e]["yt"][:, :idx.size].T  # [n_e, D]
        final[idx] += ye * w_list[e][:, None]

    return (final.reshape(B, S, D),
            router_logits,
            np.float32(bl_loss))


# revision 37
# speedup vs baseline: 1.2537x; 1.0217x over previous
"""MoE (top-2 of 8 experts, SwiGLU) Trainium2 kernel — expert-parallel over 8 NeuronCores.

Strategy
--------
- Host computes the tiny router (T x D @ D x 8 = 0.03% of total FLOPs) with the
  exact same jax ops as the reference, so top-k selection matches bitwise.
- Token dispatch ("all-to-all") happens on the host: tokens routed to expert e
  are gathered (transposed, capacity-padded) and shipped to core e.
- Each of the 8 cores runs an identical SPMD Bass program: the SwiGLU expert
  FFN for its expert over its capacity-C token slab.  Matmuls use float32r
  (full fp32 data; streams at 1 cycle/row for moving dim >= 256, i.e. bf16
  speed with fp32-grade precision).
- Host scatter-adds the two expert outputs per token back together with the
  routing weights (exactly the reference's dense-combine math restricted to
  the nonzero entries).

Per-core device work: ~C x (3 matmuls of [*,1024]x[1024,1024]) ≈ 2.6e10 FLOPs
(sparse: 4x less than the reference's dense form), vs 1.03e11 dense.
"""

import math
from contextlib import ExitStack

import numpy as np

import concourse.bass as bass
import concourse.tile as tile
from concourse import bacc, mybir
from concourse.bass_utils import run_bass_kernel_spmd

D = 1024       # model dim
H = 1024       # per-expert hidden dim
E = 8          # experts == cores
TG = 512       # tokens per tile chunk (full PSUM bank; float32r full rate >= 256)
MM_DT = mybir.dt.float32r   # moving operand (activations)
W_DT = mybir.dt.float32r    # stationary operand (weights; must match moving dtype)
ACC_DT = mybir.dt.float32
# HW has a Silu LUT (one ACT op, one DVE mul); CoreSim only implements
# Sigmoid (one ACT op, two DVE muls).  Tests flip this off to simulate.
USE_SILU = True

_BUILD_CACHE: dict[int, object] = {}


def _chunks(C):
    """Split C tokens into chunks of 512 plus one ragged 128-multiple tail."""
    out = [TG] * (C // TG)
    if C % TG:
        out.append(C % TG)
    return out


def _build(C):
    """Build + compile the single-expert SwiGLU FFN program for capacity C.

    Computes yt = (silu(xt.T @ pw) * (xt.T @ gw)) @ ow, transposed:
    everything is laid out [feature, token] so no on-device transposes are
    needed (host ships x pre-transposed and un-transposes y).
    """
    assert C % 128 == 0
    nc = bacc.Bacc("TRN2", target_bir_lowering=False, debug=False, num_devices=E)
    xt = nc.dram_tensor("xt", [D, C], MM_DT, kind="ExternalInput").ap()
    gw = nc.dram_tensor("gw", [D, H], W_DT, kind="ExternalInput").ap()
    pw = nc.dram_tensor("pw", [D, H], W_DT, kind="ExternalInput").ap()
    ow = nc.dram_tensor("ow", [H, D], W_DT, kind="ExternalInput").ap()
    yt = nc.dram_tensor("yt", [D, C], ACC_DT, kind="ExternalOutput").ap()

    KB = D // 128  # contraction blocks for the first matmuls
    HB = H // 128  # hidden blocks

    # Partition-blocked 3D views: [(a p) m] -> [p, a, m] so each weight
    # matrix / token chunk moves as ONE large DMA (HWDGE trigger is ~600ns
    # of engine time each; batching to >=1MiB is the documented rule).
    xt_r = xt.rearrange("(a p) c -> p a c", p=128)
    yt_r = yt.rearrange("(a p) c -> p a c", p=128)
    gw_r = gw.rearrange("(a p) m -> p a m", p=128)
    pw_r = pw.rearrange("(a p) m -> p a m", p=128)
    ow_r = ow.rearrange("(a p) m -> p a m", p=128)

    with tile.TileContext(nc) as tc, ExitStack() as ctx:
        wpool = ctx.enter_context(tc.tile_pool(name="w", bufs=1))
        xpool = ctx.enter_context(tc.tile_pool(name="x", bufs=2))
        hpool = ctx.enter_context(tc.tile_pool(name="h", bufs=2))
        spool = ctx.enter_context(tc.tile_pool(name="s", bufs=2))
        ypool = ctx.enter_context(tc.tile_pool(name="y", bufs=1))
        gpsum = ctx.enter_context(tc.tile_pool(name="pg", bufs=3, space="PSUM"))
        ppsum = ctx.enter_context(tc.tile_pool(name="pp", bufs=3, space="PSUM"))
        ypsum = ctx.enter_context(tc.tile_pool(name="py", bufs=2, space="PSUM"))

        def load_x(col, tg):
            t = xpool.tile([128, KB, tg], MM_DT, tag="x")
            nc.sync.dma_start(t[:], xt_r[:, :, col:col + tg])
            return t

        def load_w(ap_r, prefix):
            # two halves: G can start after the first 4 k-blocks land, and
            # the trigger pipeline (one HWDGE ring) interleaves better.
            t = wpool.tile([128, KB, H], W_DT, tag=prefix)
            half = KB // 2
            nc.sync.dma_start(t[:, :half, :], ap_r[:, :half, :])
            nc.sync.dma_start(t[:, half:, :], ap_r[:, half:, :])
            return t

        # Issue order matters: PE starts on chunk 0's G matmuls as soon as
        # x(chunk0) + gw arrive; pw is needed ~14us later, ow ~28us later
        # (and the Y phase runs one chunk behind to hide ow's wire time).
        # x0/gw/pw are split per k-tile, interleaved in exactly the order
        # chunk 0's k-outer waves consume them, so PE unblocks tile-by-tile
        # at wire rate instead of waiting for whole matrices.
        chunk_list = _chunks(C)
        tg0 = chunk_list[0]
        xs_next = xpool.tile([128, KB, tg0], MM_DT, tag="x")
        gw_t = wpool.tile([128, KB, H], W_DT, tag="gw")
        pw_t = wpool.tile([128, KB, H], W_DT, tag="pw")
        for k in range(KB):
            # x0 triggers on the (startup-idle) ACT ring, weights on SyncE's:
            # the two trigger pipelines run in parallel instead of 16 deep.
            nc.scalar.dma_start(xs_next[:, k, :], xt_r[:, k, 0:tg0])
            nc.sync.dma_start(gw_t[:, k, :], gw_r[:, k, :])
        for k in range(KB):
            nc.sync.dma_start(pw_t[:, k, :], pw_r[:, k, :])
        ow_t = load_w(ow_r, "ow")

        pending_y = None  # (hs, col, tg) of the previous chunk

        def silu_mul(pp_t, g_t, tg, h):
            """ht = g * silu(p).  g_t may be PSUM or SBUF; pp_t is PSUM —
            every DVE op reads at most one PSUM operand."""
            if USE_SILU:
                sg = spool.tile([128, tg], ACC_DT, tag="sig", name="sg")
                nc.scalar.activation(
                    sg[:], pp_t[:], mybir.ActivationFunctionType.Silu)
                ht = hpool.tile([128, tg], MM_DT, tag=f"h{h}", name="ht")
                nc.vector.tensor_mul(ht[:], g_t[:], sg[:])
            else:
                sg = spool.tile([128, tg], ACC_DT, tag="sig", name="sg")
                nc.scalar.activation(
                    sg[:], pp_t[:], mybir.ActivationFunctionType.Sigmoid)
                sl = spool.tile([128, tg], ACC_DT, tag="sil", name="sl")
                nc.vector.tensor_mul(sl[:], pp_t[:], sg[:])
                ht = hpool.tile([128, tg], MM_DT, tag=f"h{h}", name="ht")
                nc.vector.tensor_mul(ht[:], g_t[:], sl[:])
            return ht

        def emit_y(hs, ycol, tg):
            # Two half-slabs with independent DMAs: the next chunk's copies
            # only wait on the matching HALF's DRAM write, and the first
            # half's DMA fires 4 d-blocks earlier (keeps PE warm at the tail).
            half = KB // 2
            ysb = [ypool.tile([128, half, tg], ACC_DT, tag="y0", name="y0"),
                   ypool.tile([128, half, tg], ACC_DT, tag="y1", name="y1")]
            for dblk in range(KB):
                py = ypsum.tile([128, tg], ACC_DT, tag="py")
                for h in range(HB):
                    nc.tensor.matmul(
                        py[:], ow_t[:, h, dblk * 128:(dblk + 1) * 128], hs[h][:],
                        start=(h == 0), stop=(h == HB - 1))
                nc.vector.tensor_copy(ysb[dblk // half][:, dblk % half, :], py[:])
                if dblk == half - 1:
                    # ACT's HWDGE ring: keep SyncE's ring free for inputs
                    nc.scalar.dma_start(
                        yt_r[:, :half, ycol:ycol + tg], ysb[0][:])
            nc.scalar.dma_start(yt_r[:, half:, ycol:ycol + tg], ysb[1][:])

        def emit_gp_chunk0(xs, tg):
            """Chunk 0 with k-OUTER matmul order: h-outer needs the whole
            weight matrix within ~2us, but the wire delivers it over ~17us.
            All 8 h-accumulators live at once (3 pg + 3 pp + 2 py banks =
            the full 8 PSUM banks), so each k-step consumes (x0[k], gw[k])
            right as it lands — no second pass stuck behind wire waits in
            PE program order.  G results are copied to SBUF so the P phase
            can reuse the banks (silu needs P of the same h before G could
            otherwise drain)."""
            hs = []

            def bank8(prefix):
                return (
                    [gpsum.tile([128, tg], ACC_DT, tag="pg", name=f"{prefix}a{i}")
                     for i in range(3)]
                    + [ppsum.tile([128, tg], ACC_DT, tag="pp", name=f"{prefix}b{i}")
                       for i in range(3)]
                    + [ypsum.tile([128, tg], ACC_DT, tag="py", name=f"{prefix}c{i}")
                       for i in range(2)])

            pgs = bank8("g")
            for k in range(KB):
                for h in range(HB):
                    nc.tensor.matmul(
                        pgs[h][:], gw_t[:, k, h * 128:(h + 1) * 128],
                        xs[:, k, :], start=(k == 0), stop=(k == KB - 1))
            g_sb = []
            for h in range(HB):
                g = wpool.tile([128, tg], ACC_DT, tag=f"gsb{h}", name="g")
                nc.vector.tensor_copy(g[:], pgs[h][:])
                g_sb.append(g)
            # P phase in two 4-bank waves from pg/pp slots only: pw is fully
            # resident after wave 0, so wave 1 runs back-to-back, and the py
            # banks stay free for the first Y phase (avoids a pipeline-fill
            # stall that re-throttles the PE clock).
            for wave in range(2):
                pps = [gpsum.tile([128, tg], ACC_DT, tag="pg", name=f"p{wave}a0"),
                       gpsum.tile([128, tg], ACC_DT, tag="pg", name=f"p{wave}a1"),
                       ppsum.tile([128, tg], ACC_DT, tag="pp", name=f"p{wave}b0"),
                       ppsum.tile([128, tg], ACC_DT, tag="pp", name=f"p{wave}b1")]
                for k in range(KB):
                    for i in range(4):
                        h = wave * 4 + i
                        nc.tensor.matmul(
                            pps[i][:], pw_t[:, k, h * 128:(h + 1) * 128],
                            xs[:, k, :], start=(k == 0), stop=(k == KB - 1))
                for i in range(4):
                    h = wave * 4 + i
                    hs.append(silu_mul(pps[i], g_sb[h], tg, h))
            return hs

        def emit_gp(xs, tg):
            hs = []
            for h in range(HB):
                pg = gpsum.tile([128, tg], ACC_DT, tag="pg")
                for k in range(KB):
                    nc.tensor.matmul(
                        pg[:], gw_t[:, k, h * 128:(h + 1) * 128], xs[:, k, :],
                        start=(k == 0), stop=(k == KB - 1))
                pp = ppsum.tile([128, tg], ACC_DT, tag="pp")
                for k in range(KB):
                    nc.tensor.matmul(
                        pp[:], pw_t[:, k, h * 128:(h + 1) * 128], xs[:, k, :],
                        start=(k == 0), stop=(k == KB - 1))
                hs.append(silu_mul(pp, pg, tg, h))
            return hs

        col = 0
        for ci, tg in enumerate(chunk_list):
            xs = xs_next
            if ci + 1 < len(chunk_list):
                xs_next = load_x(col + tg, chunk_list[ci + 1])

            hs = emit_gp_chunk0(xs, tg) if ci == 0 else emit_gp(xs, tg)

            if pending_y is not None:
                emit_y(*pending_y)
            pending_y = (hs, col, tg)
            col += tg
        emit_y(*pending_y)

    nc.compile()
    return nc


def _get_program(C):
    if C not in _BUILD_CACHE:
        _BUILD_CACHE[C] = _build(C)
    return _BUILD_CACHE[C]


def _route(x, gate_w, k):
    """Router with the reference's exact jax ops (bitwise-matching top-k)."""
    import jax
    import jax.numpy as jnp

    router_logits = jnp.asarray(x) @ jnp.asarray(gate_w)
    routing_probs = jax.nn.softmax(router_logits.astype(jnp.float32), axis=-1)
    top_w, top_i = jax.lax.top_k(routing_probs, k)
    top_w = top_w / jnp.sum(top_w, axis=-1, keepdims=True)
    top_w = top_w.astype(jnp.float32)

    n_exp = gate_w.shape[1]
    expert_mask = jax.nn.one_hot(top_i, n_exp, dtype=jnp.float32)
    tokens_per_expert = jnp.mean(expert_mask, axis=0)
    router_prob_per_expert = jnp.mean(routing_probs, axis=0)
    bl_loss = jnp.sum(tokens_per_expert * router_prob_per_expert[None, :]) * n_exp

    return (np.asarray(router_logits), np.asarray(top_i), np.asarray(top_w),
            np.asarray(bl_loss))


def _run_device(nc, in_maps, trace=False, **kw):
    return run_bass_kernel_spmd(nc, in_maps, core_ids=list(range(E)),
                                trace=trace, **kw)


def kernel(hidden_states, gate_w, gw, pw, ow, top_k, _trace=False, _res_out=None):
    hidden_states = np.asarray(hidden_states, dtype=np.float32)
    gate_w = np.asarray(gate_w, dtype=np.float32)
    gw = np.ascontiguousarray(np.asarray(gw, dtype=np.float32))
    pw = np.ascontiguousarray(np.asarray(pw, dtype=np.float32))
    ow = np.ascontiguousarray(np.asarray(ow, dtype=np.float32))
    k = int(top_k)

    B, S, _ = hidden_states.shape
    x = hidden_states.reshape(-1, D)
    T = x.shape[0]

    w_np = mybir.dt.np(W_DT)
    if gw.dtype != w_np:
        gw, pw, ow = (a.astype(w_np) for a in (gw, pw, ow))

    router_logits, top_i, top_w, bl_loss = _route(x, gate_w, k)

    # --- host dispatch (the "all-to-all") ---
    counts = np.bincount(top_i.ravel(), minlength=E)
    C = max(128, int(math.ceil(counts.max() / 128)) * 128)
    prog = _get_program(C)

    xT = np.ascontiguousarray(x.T)  # [D, T] so per-expert gathers are column slices
    idx_list, w_list, in_maps = [], [], []
    for e in range(E):
        sel = top_i == e                       # [T, k]
        idx = np.nonzero(sel.any(axis=1))[0]   # tokens routed to e
        wgt = (top_w[idx] * sel[idx]).sum(axis=1).astype(np.float32)
        xt = np.zeros((D, C), np.float32)
        xt[:, :idx.size] = xT[:, idx]
        idx_list.append(idx)
        w_list.append(wgt)
        in_maps.append({"xt": xt, "gw": gw[e], "pw": pw[e], "ow": ow[e]})

    res = _run_device(prog, in_maps, trace=_trace)
    if _res_out is not None:
        _res_out.append(res)

    final = np.zeros((T, D), np.float32)
    for e in range(E):
        idx = idx_list[e]
        ye = res.results[e]["yt"][:, :idx.size].T  # [n_e, D]
        final[idx] += ye * w_list[e][:, None]

    return (final.reshape(B, S, D),
            router_logits,
            np.float32(bl_loss))


# revision 41
# speedup vs baseline: 1.2609x; 1.0058x over previous
"""MoE (top-2 of 8 experts, SwiGLU) Trainium2 kernel — expert-parallel over 8 NeuronCores.

Strategy
--------
- Host computes the tiny router (T x D @ D x 8 = 0.03% of total FLOPs) with the
  exact same jax ops as the reference, so top-k selection matches bitwise.
- Token dispatch ("all-to-all") happens on the host: tokens routed to expert e
  are gathered (transposed, capacity-padded) and shipped to core e.
- Each of the 8 cores runs an identical SPMD Bass program: the SwiGLU expert
  FFN for its expert over its capacity-C token slab.  Matmuls use float32r
  (full fp32 data; streams at 1 cycle/row for moving dim >= 256, i.e. bf16
  speed with fp32-grade precision).
- Host scatter-adds the two expert outputs per token back together with the
  routing weights (exactly the reference's dense-combine math restricted to
  the nonzero entries).

Per-core device work: ~C x (3 matmuls of [*,1024]x[1024,1024]) ≈ 2.6e10 FLOPs
(sparse: 4x less than the reference's dense form), vs 1.03e11 dense.
"""

import math
from contextlib import ExitStack

import numpy as np

import concourse.bass as bass
import concourse.tile as tile
from concourse import bacc, mybir
from concourse.bass_utils import run_bass_kernel_spmd

D = 1024       # model dim
H = 1024       # per-expert hidden dim
E = 8          # experts == cores
TG = 512       # tokens per tile chunk (full PSUM bank; float32r full rate >= 256)
MM_DT = mybir.dt.float32r   # moving operand (activations)
W_DT = mybir.dt.float32r    # stationary operand (weights; must match moving dtype)
ACC_DT = mybir.dt.float32
# HW has a Silu LUT (one ACT op, one DVE mul); CoreSim only implements
# Sigmoid (one ACT op, two DVE muls).  Tests flip this off to simulate.
USE_SILU = True

_BUILD_CACHE: dict[int, object] = {}


def _chunks(C):
    """Split C tokens into chunks of 512 plus one ragged 128-multiple tail."""
    out = [TG] * (C // TG)
    if C % TG:
        out.append(C % TG)
    return out


def _build(C):
    """Build + compile the single-expert SwiGLU FFN program for capacity C.

    Computes yt = (silu(xt.T @ pw) * (xt.T @ gw)) @ ow, transposed:
    everything is laid out [feature, token] so no on-device transposes are
    needed (host ships x pre-transposed and un-transposes y).
    """
    assert C % 128 == 0
    nc = bacc.Bacc("TRN2", target_bir_lowering=False, debug=False, num_devices=E)
    xt = nc.dram_tensor("xt", [D, C], MM_DT, kind="ExternalInput").ap()
    gw = nc.dram_tensor("gw", [D, H], W_DT, kind="ExternalInput").ap()
    pw = nc.dram_tensor("pw", [D, H], W_DT, kind="ExternalInput").ap()
    ow = nc.dram_tensor("ow", [H, D], W_DT, kind="ExternalInput").ap()
    yt = nc.dram_tensor("yt", [D, C], ACC_DT, kind="ExternalOutput").ap()

    KB = D // 128  # contraction blocks for the first matmuls
    HB = H // 128  # hidden blocks

    # Partition-blocked 3D views: [(a p) m] -> [p, a, m] so each weight
    # matrix / token chunk moves as ONE large DMA (HWDGE trigger is ~600ns
    # of engine time each; batching to >=1MiB is the documented rule).
    xt_r = xt.rearrange("(a p) c -> p a c", p=128)
    yt_r = yt.rearrange("(a p) c -> p a c", p=128)
    gw_r = gw.rearrange("(a p) m -> p a m", p=128)
    pw_r = pw.rearrange("(a p) m -> p a m", p=128)
    ow_r = ow.rearrange("(a p) m -> p a m", p=128)

    with tile.TileContext(nc) as tc, ExitStack() as ctx:
        wpool = ctx.enter_context(tc.tile_pool(name="w", bufs=1))
        xpool = ctx.enter_context(tc.tile_pool(name="x", bufs=3))
        hpool = ctx.enter_context(tc.tile_pool(name="h", bufs=2))
        spool = ctx.enter_context(tc.tile_pool(name="s", bufs=2))
        ypool = ctx.enter_context(tc.tile_pool(name="y", bufs=1))
        gpsum = ctx.enter_context(tc.tile_pool(name="pg", bufs=3, space="PSUM"))
        ppsum = ctx.enter_context(tc.tile_pool(name="pp", bufs=3, space="PSUM"))
        ypsum = ctx.enter_context(tc.tile_pool(name="py", bufs=2, space="PSUM"))

        def load_x(col, tg):
            t = xpool.tile([128, KB, tg], MM_DT, tag="x")
            nc.sync.dma_start(t[:], xt_r[:, :, col:col + tg])
            return t

        def load_w(ap_r, prefix):
            # two halves: G can start after the first 4 k-blocks land, and
            # the trigger pipeline (one HWDGE ring) interleaves better.
            t = wpool.tile([128, KB, H], W_DT, tag=prefix)
            half = KB // 2
            nc.sync.dma_start(t[:, :half, :], ap_r[:, :half, :])
            nc.sync.dma_start(t[:, half:, :], ap_r[:, half:, :])
            return t

        # Issue order matters: PE starts on chunk 0's G matmuls as soon as
        # x(chunk0) + gw arrive; pw is needed ~14us later, ow ~28us later
        # (and the Y phase runs one chunk behind to hide ow's wire time).
        # x0/gw/pw are split per k-tile, interleaved in exactly the order
        # chunk 0's k-outer waves consume them, so PE unblocks tile-by-tile
        # at wire rate instead of waiting for whole matrices.
        chunk_list = _chunks(C)
        tg0 = chunk_list[0]
        xs_next = xpool.tile([128, KB, tg0], MM_DT, tag="x")
        gw_t = wpool.tile([128, KB, H], W_DT, tag="gw")
        pw_t = wpool.tile([128, KB, H], W_DT, tag="pw")
        for k in range(KB):
            # x0 triggers on the (startup-idle) ACT ring, weights on SyncE's:
            # the two trigger pipelines run in parallel instead of 16 deep.
            nc.scalar.dma_start(xs_next[:, k, :], xt_r[:, k, 0:tg0])
            nc.sync.dma_start(gw_t[:, k, :], gw_r[:, k, :])
        for k in range(KB):
            nc.sync.dma_start(pw_t[:, k, :], pw_r[:, k, :])
        # x(chunk1) before ow: it's consumed ~25us earlier than ow is.
        xq = [xs_next]
        if len(chunk_list) > 1:
            xq.append(load_x(tg0, chunk_list[1]))
        ow_t = load_w(ow_r, "ow")

        pending_y = None  # (hs, col, tg) of the previous chunk

        def silu_mul(pp_t, g_t, tg, h):
            """ht = g * silu(p).  g_t may be PSUM or SBUF; pp_t is PSUM —
            every DVE op reads at most one PSUM operand."""
            if USE_SILU:
                sg = spool.tile([128, tg], ACC_DT, tag="sig", name="sg")
                nc.scalar.activation(
                    sg[:], pp_t[:], mybir.ActivationFunctionType.Silu)
                ht = hpool.tile([128, tg], MM_DT, tag=f"h{h}", name="ht")
                nc.vector.tensor_mul(ht[:], g_t[:], sg[:])
            else:
                sg = spool.tile([128, tg], ACC_DT, tag="sig", name="sg")
                nc.scalar.activation(
                    sg[:], pp_t[:], mybir.ActivationFunctionType.Sigmoid)
                sl = spool.tile([128, tg], ACC_DT, tag="sil", name="sl")
                nc.vector.tensor_mul(sl[:], pp_t[:], sg[:])
                ht = hpool.tile([128, tg], MM_DT, tag=f"h{h}", name="ht")
                nc.vector.tensor_mul(ht[:], g_t[:], sl[:])
            return ht

        def emit_y(hs, ycol, tg):
            # Two half-slabs with independent DMAs: the next chunk's copies
            # only wait on the matching HALF's DRAM write, and the first
            # half's DMA fires 4 d-blocks earlier (keeps PE warm at the tail).
            half = KB // 2
            ysb = [ypool.tile([128, half, tg], ACC_DT, tag="y0", name="y0"),
                   ypool.tile([128, half, tg], ACC_DT, tag="y1", name="y1")]
            for dblk in range(KB):
                py = ypsum.tile([128, tg], ACC_DT, tag="py")
                for h in range(HB):
                    nc.tensor.matmul(
                        py[:], ow_t[:, h, dblk * 128:(dblk + 1) * 128], hs[h][:],
                        start=(h == 0), stop=(h == HB - 1))
                nc.vector.tensor_copy(ysb[dblk // half][:, dblk % half, :], py[:])
                if dblk == half - 1:
                    # ACT's HWDGE ring: keep SyncE's ring free for inputs
                    nc.scalar.dma_start(
                        yt_r[:, :half, ycol:ycol + tg], ysb[0][:])
            nc.scalar.dma_start(yt_r[:, half:, ycol:ycol + tg], ysb[1][:])

        def emit_gp_chunk0(xs, tg):
            """Chunk 0 with k-OUTER matmul order: h-outer needs the whole
            weight matrix within ~2us, but the wire delivers it over ~17us.
            All 8 h-accumulators live at once (3 pg + 3 pp + 2 py banks =
            the full 8 PSUM banks), so each k-step consumes (x0[k], gw[k])
            right as it lands — no second pass stuck behind wire waits in
            PE program order.  G results are copied to SBUF so the P phase
            can reuse the banks (silu needs P of the same h before G could
            otherwise drain)."""
            hs = []

            def bank8(prefix):
                return (
                    [gpsum.tile([128, tg], ACC_DT, tag="pg", name=f"{prefix}a{i}")
                     for i in range(3)]
                    + [ppsum.tile([128, tg], ACC_DT, tag="pp", name=f"{prefix}b{i}")
                       for i in range(3)]
                    + [ypsum.tile([128, tg], ACC_DT, tag="py", name=f"{prefix}c{i}")
                       for i in range(2)])

            pgs = bank8("g")
            for k in range(KB):
                for h in range(HB):
                    nc.tensor.matmul(
                        pgs[h][:], gw_t[:, k, h * 128:(h + 1) * 128],
                        xs[:, k, :], start=(k == 0), stop=(k == KB - 1))
            # Stage G in the y0/y1 slabs: they are idle until the first Y
            # phase (~55us), long after these are consumed (~35us) — saves
            # 16KB/partition of SBUF.
            half = KB // 2
            slab = [ypool.tile([128, half, tg], ACC_DT, tag="y0", name="gs0"),
                    ypool.tile([128, half, tg], ACC_DT, tag="y1", name="gs1")]
            g_sb = []
            for h in range(HB):
                g = slab[h // half][:, h % half, :]
                nc.vector.tensor_copy(g[:], pgs[h][:])
                g_sb.append(g)
            # P phase in two 4-bank waves from pg/pp slots only: pw is fully
            # resident after wave 0, so wave 1 runs back-to-back, and the py
            # banks stay free for the first Y phase (avoids a pipeline-fill
            # stall that re-throttles the PE clock).
            for wave in range(2):
                pps = [gpsum.tile([128, tg], ACC_DT, tag="pg", name=f"p{wave}a0"),
                       gpsum.tile([128, tg], ACC_DT, tag="pg", name=f"p{wave}a1"),
                       ppsum.tile([128, tg], ACC_DT, tag="pp", name=f"p{wave}b0"),
                       ppsum.tile([128, tg], ACC_DT, tag="pp", name=f"p{wave}b1")]
                for k in range(KB):
                    for i in range(4):
                        h = wave * 4 + i
                        nc.tensor.matmul(
                            pps[i][:], pw_t[:, k, h * 128:(h + 1) * 128],
                            xs[:, k, :], start=(k == 0), stop=(k == KB - 1))
                for i in range(4):
                    h = wave * 4 + i
                    hs.append(silu_mul(pps[i], g_sb[h], tg, h))
            return hs

        def emit_gp(xs, tg):
            hs = []
            for h in range(HB):
                pg = gpsum.tile([128, tg], ACC_DT, tag="pg")
                for k in range(KB):
                    nc.tensor.matmul(
                        pg[:], gw_t[:, k, h * 128:(h + 1) * 128], xs[:, k, :],
                        start=(k == 0), stop=(k == KB - 1))
                pp = ppsum.tile([128, tg], ACC_DT, tag="pp")
                for k in range(KB):
                    nc.tensor.matmul(
                        pp[:], pw_t[:, k, h * 128:(h + 1) * 128], xs[:, k, :],
                        start=(k == 0), stop=(k == KB - 1))
                hs.append(silu_mul(pp, pg, tg, h))
            return hs

        col = 0
        for ci, tg in enumerate(chunk_list):
            xs = xq.pop(0)
            if ci + 2 < len(chunk_list):   # keep the prefetch queue 2 deep
                xq.append(load_x(sum(chunk_list[:ci + 2]), chunk_list[ci + 2]))

            hs = emit_gp_chunk0(xs, tg) if ci == 0 else emit_gp(xs, tg)

            if pending_y is not None:
                emit_y(*pending_y)
            pending_y = (hs, col, tg)
            col += tg
        emit_y(*pending_y)

    nc.compile()
    return nc


def _get_program(C):
    if C not in _BUILD_CACHE:
        _BUILD_CACHE[C] = _build(C)
    return _BUILD_CACHE[C]


def _route(x, gate_w, k):
    """Router with the reference's exact jax ops (bitwise-matching top-k)."""
    import jax
    import jax.numpy as jnp

    router_logits = jnp.asarray(x) @ jnp.asarray(gate_w)
    routing_probs = jax.nn.softmax(router_logits.astype(jnp.float32), axis=-1)
    top_w, top_i = jax.lax.top_k(routing_probs, k)
    top_w = top_w / jnp.sum(top_w, axis=-1, keepdims=True)
    top_w = top_w.astype(jnp.float32)

    n_exp = gate_w.shape[1]
    expert_mask = jax.nn.one_hot(top_i, n_exp, dtype=jnp.float32)
    tokens_per_expert = jnp.mean(expert_mask, axis=0)
    router_prob_per_expert = jnp.mean(routing_probs, axis=0)
    bl_loss = jnp.sum(tokens_per_expert * router_prob_per_expert[None, :]) * n_exp

    return (np.asarray(router_logits), np.asarray(top_i), np.asarray(top_w),
            np.asarray(bl_loss))


def _run_device(nc, in_maps, trace=False, **kw):
    return run_bass_kernel_spmd(nc, in_maps, core_ids=list(range(E)),
                                trace=trace, **kw)


def kernel(hidden_states, gate_w, gw, pw, ow, top_k, _trace=False, _res_out=None):
    hidden_states = np.asarray(hidden_states, dtype=np.float32)
    gate_w = np.asarray(gate_w, dtype=np.float32)
    gw = np.ascontiguousarray(np.asarray(gw, dtype=np.float32))
    pw = np.ascontiguousarray(np.asarray(pw, dtype=np.float32))
    ow = np.ascontiguousarray(np.asarray(ow, dtype=np.float32))
    k = int(top_k)

    B, S, _ = hidden_states.shape
    x = hidden_states.reshape(-1, D)
    T = x.shape[0]

    w_np = mybir.dt.np(W_DT)
    if gw.dtype != w_np:
        gw, pw, ow = (a.astype(w_np) for a in (gw, pw, ow))

    router_logits, top_i, top_w, bl_loss = _route(x, gate_w, k)

    # --- host dispatch (the "all-to-all") ---
    counts = np.bincount(top_i.ravel(), minlength=E)
    C = max(128, int(math.ceil(counts.max() / 128)) * 128)
    prog = _get_program(C)

    xT = np.ascontiguousarray(x.T)  # [D, T] so per-expert gathers are column slices
    idx_list, w_list, in_maps = [], [], []
    for e in range(E):
        sel = top_i == e                       # [T, k]
        idx = np.nonzero(sel.any(axis=1))[0]   # tokens routed to e
        wgt = (top_w[idx] * sel[idx]).sum(axis=1).astype(np.float32)
        xt = np.zeros((D, C), np.float32)
        xt[:, :idx.size] = xT[:, idx]
        idx_list.append(idx)
        w_list.append(wgt)
        in_maps.append({"xt": xt, "gw": gw[e], "pw": pw[e], "ow": ow[e]})

    res = _run_device(prog, in_maps, trace=_trace)
    if _res_out is not None:
        _res_out.append(res)

    final = np.zeros((T, D), np.float32)
    for e in range(E):
        idx = idx_list[e]
        ye = res.results[e]["yt"][:, :idx.size].T  # [n_e, D]
        final[idx] += ye * w_list[e][:, None]

    return (final.reshape(B, S, D),
            router_logits,
            np.float32(bl_loss))
